# revision 1
# baseline (speedup 1.0000x reference)
"""Trainium2 Bass kernel for nn_AttentionTD (3-block deformable attention TD).

Self-contained: hardcodes all shapes. Data-parallel over batch B=8 across the
8 NeuronCores; each core runs the full 3-block DAT stack for one batch element.
"""

import sys

sys.path.insert(0, "/opt/trn_rl_repo")

import numpy as np

# ---------------- problem constants ----------------
B, C, H, W = 8, 128, 64, 64
NCH = 64          # channels per DAT block
NH, HC = 4, 16    # heads, head channels
KS = 4
HWS = H * W       # 4096
HK = WK = 16
NS = HK * WK      # 256 sample points
EPS = 1e-5
NBLK = 3
# rpe slice table geometry: [blk][h][x0 (64)][row (128)][col (65)]
TROW, TCOL = 128, 65
TSLICE = TROW * TCOL          # 8320
THEAD = 64 * TSLICE           # per (blk,h)
TBLK = NH * THEAD
NTAB = NBLK * TBLK

_CACHE = {}


def _build_graph(dbg=False):
    from concourse import bacc, mybir, tile
    import concourse.bass as bass
    from concourse.bass import IndirectOffsetOnAxis

    f32 = mybir.dt.float32
    bf16 = mybir.dt.bfloat16
    i32 = mybir.dt.int32
    Alu = mybir.AluOpType
    Act = mybir.ActivationFunctionType

    nc = bacc.Bacc("TRN2", target_bir_lowering=False, debug=False, num_devices=8)

    # ---- dram io ----
    xi1_d = nc.dram_tensor("xi1", [C, HWS], f32, kind="ExternalInput").ap()
    xi2_d = nc.dram_tensor("xi2", [C, HWS], f32, kind="ExternalInput").ap()
    kvT0_d = nc.dram_tensor("kvT0", [HWS, NCH], f32, kind="ExternalInput").ap()
    kvT1_d = nc.dram_tensor("kvT1", [HWS, NCH], f32, kind="ExternalInput").ap()
    wpf_d = nc.dram_tensor("wpf", [64, 3 * 128], f32, kind="ExternalInput").ap()
    wpb_d = nc.dram_tensor("wpb", [65, 3 * 192], bf16, kind="ExternalInput").ap()
    cp_d = nc.dram_tensor("cp", [128, 590], f32, kind="ExternalInput").ap()
    cpb_d = nc.dram_tensor("cpb", [128, 320], bf16, kind="ExternalInput").ap()
    tab_d = nc.dram_tensor("rpetab", [2 * NTAB, 1], bf16, kind="ExternalInput").ap()
    o1_d = nc.dram_tensor("o1", [C, HWS], f32, kind="ExternalOutput").ap()
    o2_d = nc.dram_tensor("o2", [C, HWS], f32, kind="ExternalOutput").ap()
    dbg_d = nc.dram_tensor("dbg", [2, 4096], f32, kind="ExternalOutput").ap() if dbg else None

    with tile.TileContext(nc) as tc:
        import contextlib

        ctx = contextlib.ExitStack()
        with ctx:
            cpool = ctx.enter_context(tc.tile_pool(name="const", bufs=1))
            xpool = ctx.enter_context(tc.tile_pool(name="xdata", bufs=1))
            qpool = ctx.enter_context(tc.tile_pool(name="qtiles", bufs=2))
            sb = ctx.enter_context(tc.tile_pool(name="work", bufs=2))
            sbs = ctx.enter_context(tc.tile_pool(name="small", bufs=2))
            wpool = ctx.enter_context(tc.tile_pool(name="wins", bufs=2))
            ppool = ctx.enter_context(tc.tile_pool(name="probs", bufs=1))
            qkps = ctx.enter_context(tc.tile_pool(name="qk", bufs=3, space="PSUM"))
            avps = ctx.enter_context(tc.tile_pool(name="av", bufs=2, space="PSUM"))
            mps = ctx.enter_context(tc.tile_pool(name="misc", bufs=1, space="PSUM"))
            tps = ctx.enter_context(tc.tile_pool(name="tailp", bufs=2, space="PSUM"))

            # ---- persistent loads ----
            cp = cpool.tile([128, 590], f32, tag="cp")
            nc.sync.dma_start(out=cp[:, :], in_=cp_d)
            wpf = cpool.tile([64, 3 * 128], f32, tag="wpf")
            nc.sync.dma_start(out=wpf[:, :], in_=wpf_d)
            wpb = cpool.tile([65, 3 * 192], bf16, tag="wpb")
            nc.sync.dma_start(out=wpb[:, :], in_=wpb_d)
            cpb = cpool.tile([128, 320], bf16, tag="cpb")
            nc.sync.dma_start(out=cpb[:, :], in_=cpb_d)
            xi1 = xpool.tile([C, HWS], f32, tag="xi1")
            nc.sync.dma_start(out=xi1[:, :], in_=xi1_d)
            xi2 = xpool.tile([C, HWS], f32, tag="xi2")
            nc.sync.dma_start(out=xi2[:, :], in_=xi2_d)

            def act_raw(out, in_, func):
                eng = nc.scalar
                ins = [eng.lower_ap(in_)]
                for v in (0.0, 1.0, 0.0):
                    ins.append(mybir.ImmediateValue(dtype=mybir.dt.float32, value=v))
                return eng.add_instruction(
                    mybir.InstActivation(
                        name=nc.get_next_instruction_name(), func=func,
                        ins=ins, outs=[eng.lower_ap(out)],
                    )
                )

            zb = cpool.tile([128, 1], f32, tag="zb")
            nc.vector.memset(zb[:, :], 0.0)
            epst = cpool.tile([1, 1], f32, tag="epst")
            nc.vector.memset(epst[:, :], EPS)

            eye = cp[:, 0:128]
            ref_yx = cp[0:2, 128:384]          # row0 = y, row1 = x
            ones1_128 = cp[0:1, 384:512]       # [1,128] ones (bcast lhsT)
            ones128_div = cp[0:128, 520:521]   # 1/64 on data rows, 0 on gaps

            def wf(blk, lo, hi):
                return wpf[:, blk * 128 + lo : blk * 128 + hi]

            def wb(blk, lo, hi, rows=64):
                return wpb[0:rows, blk * 192 + lo : blk * 192 + hi]

            if dbg:
                dbgt = xpool.tile([2, 4096], f32, tag="dbgt")
                nc.vector.memset(dbgt[:, :], 0.0)

            def emit_block(blk, XQ, kvT_ap, R):
                pq_wT_sp = wf(blk, 0, 128)
                bc0 = 527 + blk * 21
                dw_w = cp[:, bc0 : bc0 + 16]
                dw_b = cp[:, bc0 + 16 : bc0 + 17]
                ln_g = cp[:, bc0 + 17 : bc0 + 18]
                ln_b = cp[:, bc0 + 18 : bc0 + 19]
                pw_wT = cp[:, bc0 + 19 : bc0 + 21]
                pk_wTs1 = wb(blk, 0, 128, rows=65)
                pv_wT1 = wb(blk, 128, 192, rows=65)
                po_wT_sp = cpb[:, 128 + blk * 64 : 128 + (blk + 1) * 64]
                b4 = cpb[:, 0:128]
                pq_b_sp = cp[:, 521 + blk : 522 + blk]
                po_b_hi = cp[64:128, 524 + blk : 525 + blk]
                
                # ---------- q projection (f32, spread head layout) ----------
                q_f = qpool.tile([128, HWS], f32, tag="qf")
                q_b = qpool.tile([128, HWS], bf16, tag="qb")
                for mc in range(8):
                    qp = mps.tile([128, 512], f32, tag="m")
                    nc.tensor.matmul(
                        out=qp[:, :], lhsT=pq_wT_sp, rhs=XQ[0:64, mc * 512 : (mc + 1) * 512],
                        start=True, stop=True,
                    )
                    nc.vector.tensor_scalar(
                        out=q_f[:, mc * 512 : (mc + 1) * 512], in0=qp[:, :],
                        scalar1=pq_b_sp, scalar2=None, op0=Alu.add,
                    )
                    nc.vector.tensor_scalar(
                        out=q_b[:, mc * 512 : (mc + 1) * 512], in0=qp[:, :],
                        scalar1=pq_b_sp, scalar2=None, op0=Alu.add,
                    )

                # ---------- depthwise 4x4 stride-4 conv ----------
                q5 = q_f[:, :].rearrange("p (hh a ww b) -> p hh a ww b", hh=16, a=4, ww=16, b=4)
                acc = sbs.tile([128, NS], f32, tag="dwacc")
                nc.vector.tensor_scalar(
                    out=acc[:, :], in0=q5[:, :, 0, :, 0], scalar1=dw_w[:, 0:1],
                    scalar2=None, op0=Alu.mult,
                )
                for t in range(1, 16):
                    dy, dx = t // 4, t % 4
                    nc.vector.scalar_tensor_tensor(
                        out=acc[:, :], in0=q5[:, :, dy, :, dx],
                        scalar=dw_w[:, t : t + 1], in1=acc[:, :],
                        op0=Alu.mult, op1=Alu.add,
                    )
                nc.vector.tensor_scalar(
                    out=acc[:, :], in0=acc[:, :], scalar1=dw_b, scalar2=None, op0=Alu.add
                )

                # ---------- layernorm over channels ----------
                sq = sbs.tile([128, NS], f32, tag="sq")
                nc.vector.tensor_tensor(out=sq[:, :], in0=acc[:, :], in1=acc[:, :], op=Alu.mult)
                mu_p = mps.tile([1, NS], f32, tag="m")
                nc.tensor.matmul(out=mu_p[:, :], lhsT=ones128_div, rhs=acc[:, :], start=True, stop=True)
                e2_p = mps.tile([1, NS], f32, tag="m")
                nc.tensor.matmul(out=e2_p[:, :], lhsT=ones128_div, rhs=sq[:, :], start=True, stop=True)
                stats = sbs.tile([1, 2 * NS], f32, tag="stats")
                nc.vector.tensor_copy(out=stats[:, 0:NS], in_=mu_p[:, :])
                mu2 = sbs.tile([1, NS], f32, tag="mu2")
                nc.vector.tensor_tensor(out=mu2[:, :], in0=stats[:, 0:NS], in1=stats[:, 0:NS], op=Alu.mult)
                var = sbs.tile([1, NS], f32, tag="var")
                nc.vector.tensor_tensor(out=var[:, :], in0=e2_p[:, :], in1=mu2[:, :], op=Alu.subtract)
                sd = sbs.tile([1, NS], f32, tag="sd")
                nc.scalar.activation(out=sd[:, :], in_=var[:, :], func=Act.Sqrt, bias=epst[:, :])
                nc.vector.reciprocal(out=stats[:, NS : 2 * NS], in_=sd[:, :])
                bc_p = mps.tile([128, 2 * NS], f32, tag="m")
                nc.tensor.matmul(out=bc_p[:, :], lhsT=ones1_128, rhs=stats[:, :], start=True, stop=True)
                t1 = sbs.tile([128, NS], f32, tag="t1")
                nc.vector.tensor_tensor(out=t1[:, :], in0=acc[:, :], in1=bc_p[:, 0:NS], op=Alu.subtract)
                nc.vector.tensor_tensor(out=t1[:, :], in0=t1[:, :], in1=bc_p[:, NS : 2 * NS], op=Alu.mult)
                nc.vector.tensor_scalar(
                    out=t1[:, :], in0=t1[:, :], scalar1=ln_g, scalar2=ln_b,
                    op0=Alu.mult, op1=Alu.add,
                )
                # exact GELU via Abramowitz-Stegun erf (|err| <= 1.5e-7)
                gl = sbs.tile([128, NS], f32, tag="gl")
                ze = sbs.tile([128, NS], f32, tag="ze")
                nc.scalar.activation(out=ze[:, :], in_=t1[:, :], func=Act.Abs,
                                     bias=zb[:, :], scale=0.7071067811865476)
                tt_ = sbs.tile([128, NS], f32, tag="tt")
                nc.vector.tensor_scalar(out=tt_[:, :], in0=ze[:, :], scalar1=0.3275911,
                                        scalar2=1.0, op0=Alu.mult, op1=Alu.add)
                nc.vector.reciprocal(out=tt_[:, :], in_=tt_[:, :])
                poly = sbs.tile([128, NS], f32, tag="poly")
                A = (1.061405429, -1.453152027, 1.421413741, -0.284496736, 0.254829592)
                nc.vector.tensor_scalar(out=poly[:, :], in0=tt_[:, :], scalar1=A[0],
                                        scalar2=A[1], op0=Alu.mult, op1=Alu.add)
                for a_c in A[2:]:
                    nc.vector.tensor_tensor(out=poly[:, :], in0=poly[:, :], in1=tt_[:, :], op=Alu.mult)
                    nc.vector.tensor_scalar(out=poly[:, :], in0=poly[:, :], scalar1=a_c,
                                            scalar2=None, op0=Alu.add)
                nc.vector.tensor_tensor(out=poly[:, :], in0=poly[:, :], in1=tt_[:, :], op=Alu.mult)
                ez = sbs.tile([128, NS], f32, tag="ez")
                nc.vector.tensor_tensor(out=ez[:, :], in0=ze[:, :], in1=ze[:, :], op=Alu.mult)
                nc.scalar.activation(out=ez[:, :], in_=ez[:, :], func=Act.Exp,
                                     bias=zb[:, :], scale=-1.0)
                nc.vector.tensor_tensor(out=poly[:, :], in0=poly[:, :], in1=ez[:, :], op=Alu.mult)
                # erf_abs = 1 - poly
                nc.vector.tensor_scalar(out=poly[:, :], in0=poly[:, :], scalar1=-1.0,
                                        scalar2=1.0, op0=Alu.mult, op1=Alu.add)
                # phi = 0.5 + sign(x)*0.5*erf_abs ; gelu = x*phi
                nc.vector.tensor_tensor(out=ze[:, :], in0=t1[:, :],
                                        in1=zb[:, :].to_broadcast([128, NS]), op=Alu.is_gt)
                nc.vector.tensor_scalar(out=ze[:, :], in0=ze[:, :], scalar1=1.0,
                                        scalar2=-0.5, op0=Alu.mult, op1=Alu.add)
                nc.vector.tensor_tensor(out=poly[:, :], in0=poly[:, :], in1=ze[:, :], op=Alu.mult)
                nc.vector.tensor_scalar(out=poly[:, :], in0=poly[:, :], scalar1=0.5,
                                        scalar2=None, op0=Alu.add)
                nc.vector.tensor_tensor(out=gl[:, :], in0=t1[:, :], in1=poly[:, :], op=Alu.mult)

                # ---------- offsets -> positions ----------
                off_p = mps.tile([2, NS], f32, tag="m")
                nc.tensor.matmul(out=off_p[:, :], lhsT=pw_wT, rhs=gl[:, :], start=True, stop=True)
                pos = sbs.tile([2, NS], f32, tag="pos")
                nc.vector.tensor_tensor(out=pos[:, :], in0=off_p[:, :], in1=ref_yx, op=Alu.add)
                nc.vector.tensor_scalar(
                    out=pos[:, :], in0=pos[:, :], scalar1=1.0, scalar2=-1.0,
                    op0=Alu.min, op1=Alu.max,
                )

                if dbg:
                    nc.vector.tensor_copy(out=dbgt[0:2, blk * 256 : blk * 256 + 256], in_=pos[:, :])
                    if blk == 0:
                        nc.vector.tensor_copy(out=dbgt[0:1, 768:1280], in_=stats[:, :])
                        nc.vector.tensor_copy(out=dbgt[0:1, 1280:1536], in_=acc[0:1, :])
                        nc.vector.tensor_copy(out=dbgt[0:1, 1536:2560], in_=q_f[0:1, 0:1024])
                        nc.vector.tensor_copy(out=dbgt[0:1, 2560:2816], in_=gl[0:1, :])

                # transpose pos -> [n,(y,x)] per 128-chunk
                posT = sbs.tile([128, 4], f32, tag="posT")  # cols: c0y c0x c1y c1x
                for c in range(2):
                    tp = mps.tile([128, 2], f32, tag="m")
                    nc.tensor.transpose(
                        out=tp[:, :], in_=pos[:, c * 128 : (c + 1) * 128], identity=eye[0:2, 0:2]
                    )
                    nc.vector.tensor_copy(out=posT[:, c * 2 : c * 2 + 2], in_=tp[:, :])

                # ---------- per-chunk index & weight math ----------
                idxkv = sbs.tile([128, 8], f32, tag="idxkv")
                idxw = sbs.tile([128, 8], f32, tag="idxw")
                fyb = sbs.tile([128, 2], f32, tag="fyb")
                wkv = sbs.tile([128, 8], f32, tag="wkv")   # w00 w01 w10 w11 per chunk
                dxw = sbs.tile([128, 4], f32, tag="dxw")   # (1-fxb, fxb) per chunk
                scr = sbs.tile([128, 12], f32, tag="scr")

                for c in range(2):
                    y = posT[:, c * 2 : c * 2 + 1]
                    x = posT[:, c * 2 + 1 : c * 2 + 2]
                    # kv pixel coords
                    xf = scr[:, 0:1]
                    yf = scr[:, 1:2]
                    nc.vector.tensor_scalar(out=xf, in0=x, scalar1=1.0, scalar2=31.5, op0=Alu.add, op1=Alu.mult)
                    nc.vector.tensor_scalar(out=yf, in0=y, scalar1=1.0, scalar2=31.5, op0=Alu.add, op1=Alu.mult)
                    xm = scr[:, 2:3]
                    ym = scr[:, 3:4]
                    x0 = scr[:, 4:5]
                    y0 = scr[:, 5:6]
                    # floor via round-to-nearest (+2^23) then subtract (r > x)
                    nc.vector.tensor_scalar(out=x0, in0=xf, scalar1=8388608.0, scalar2=-8388608.0, op0=Alu.add, op1=Alu.add)
                    nc.vector.tensor_tensor(out=xm, in0=x0, in1=xf, op=Alu.is_gt)
                    nc.vector.tensor_tensor(out=x0, in0=x0, in1=xm, op=Alu.subtract)
                    nc.vector.tensor_scalar(out=x0, in0=x0, scalar1=62.0, scalar2=None, op0=Alu.min)
                    nc.vector.tensor_scalar(out=y0, in0=yf, scalar1=8388608.0, scalar2=-8388608.0, op0=Alu.add, op1=Alu.add)
                    nc.vector.tensor_tensor(out=ym, in0=y0, in1=yf, op=Alu.is_gt)
                    nc.vector.tensor_tensor(out=y0, in0=y0, in1=ym, op=Alu.subtract)
                    nc.vector.tensor_scalar(out=y0, in0=y0, scalar1=62.0, scalar2=None, op0=Alu.min)
                    fx = scr[:, 6:7]
                    fy = scr[:, 7:8]
                    nc.vector.tensor_tensor(out=fx, in0=xf, in1=x0, op=Alu.subtract)
                    nc.vector.tensor_tensor(out=fy, in0=yf, in1=y0, op=Alu.subtract)
                    fx1 = scr[:, 8:9]
                    fy1 = scr[:, 9:10]
                    nc.vector.tensor_scalar(out=fx1, in0=fx, scalar1=-1.0, scalar2=1.0, op0=Alu.mult, op1=Alu.add)
                    nc.vector.tensor_scalar(out=fy1, in0=fy, scalar1=-1.0, scalar2=1.0, op0=Alu.mult, op1=Alu.add)
                    nc.vector.tensor_tensor(out=wkv[:, c * 4 + 0 : c * 4 + 1], in0=fy1, in1=fx1, op=Alu.mult)
                    nc.vector.tensor_tensor(out=wkv[:, c * 4 + 1 : c * 4 + 2], in0=fy1, in1=fx, op=Alu.mult)
                    nc.vector.tensor_tensor(out=wkv[:, c * 4 + 2 : c * 4 + 3], in0=fy, in1=fx1, op=Alu.mult)
                    nc.vector.tensor_tensor(out=wkv[:, c * 4 + 3 : c * 4 + 4], in0=fy, in1=fx, op=Alu.mult)
                    # kv gather indices: y0*64+x0 (+0,+1,+64,+65)
                    ib = scr[:, 10:11]
                    nc.vector.scalar_tensor_tensor(out=ib, in0=y0, scalar=64.0, in1=x0, op0=Alu.mult, op1=Alu.add)
                    for t, offt in enumerate((0.0, 1.0, 64.0, 65.0)):
                        nc.vector.tensor_scalar(
                            out=idxkv[:, c * 4 + t : c * 4 + t + 1], in0=ib,
                            scalar1=offt, scalar2=None, op0=Alu.add,
                        )
                    # bias window coords: cx = 31.5*(1-x), cy = 31.5*(1-y)
                    cxf = scr[:, 0:1]
                    cyf = scr[:, 1:2]
                    nc.vector.tensor_scalar(out=cxf, in0=x, scalar1=-31.5, scalar2=31.5, op0=Alu.mult, op1=Alu.add)
                    nc.vector.tensor_scalar(out=cyf, in0=y, scalar1=-31.5, scalar2=31.5, op0=Alu.mult, op1=Alu.add)
                    fbx = scr[:, 2:3]
                    fby = scr[:, 3:4]
                    x0b = scr[:, 4:5]
                    y0b = scr[:, 5:6]
                    nc.vector.tensor_scalar(out=x0b, in0=cxf, scalar1=8388608.0, scalar2=-8388608.0, op0=Alu.add, op1=Alu.add)
                    nc.vector.tensor_tensor(out=fbx, in0=x0b, in1=cxf, op=Alu.is_gt)
                    nc.vector.tensor_tensor(out=x0b, in0=x0b, in1=fbx, op=Alu.subtract)
                    nc.vector.tensor_scalar(out=y0b, in0=cyf, scalar1=8388608.0, scalar2=-8388608.0, op0=Alu.add, op1=Alu.add)
                    nc.vector.tensor_tensor(out=fby, in0=y0b, in1=cyf, op=Alu.is_gt)
                    nc.vector.tensor_tensor(out=y0b, in0=y0b, in1=fby, op=Alu.subtract)
                    nc.vector.tensor_tensor(out=fbx, in0=cxf, in1=x0b, op=Alu.subtract)
                    nc.vector.tensor_tensor(out=fby, in0=cyf, in1=y0b, op=Alu.subtract)
                    nc.vector.tensor_copy(out=fyb[:, c : c + 1], in_=fby)
                    nc.vector.tensor_scalar(out=dxw[:, c * 2 : c * 2 + 1], in0=fbx, scalar1=-1.0, scalar2=1.0, op0=Alu.mult, op1=Alu.add)
                    nc.vector.tensor_copy(out=dxw[:, c * 2 + 1 : c * 2 + 2], in_=fbx)
                    # window index: ((x0b*128)+y0b)*65 + blk_base (+h stride via cp consts)
                    iw = scr[:, 11:12]
                    nc.vector.scalar_tensor_tensor(out=iw, in0=x0b, scalar=128.0, in1=y0b, op0=Alu.mult, op1=Alu.add)
                    nc.vector.tensor_scalar(
                        out=iw, in0=iw, scalar1=65.0, scalar2=float(blk * TBLK),
                        op0=Alu.mult, op1=Alu.add,
                    )
                    for hh in range(4):
                        nc.vector.tensor_scalar(
                            out=idxw[:, c * 4 + hh : c * 4 + hh + 1], in0=iw,
                            scalar1=float(hh * THEAD), scalar2=None, op0=Alu.add,
                        )

                idxkv_i = sbs.tile([128, 8], i32, tag="idxkvi")
                nc.vector.tensor_copy(out=idxkv_i[:, :], in_=idxkv[:, :])
                idxw_i = sbs.tile([128, 8], i32, tag="idxwi")
                nc.vector.tensor_copy(out=idxw_i[:, :], in_=idxw[:, :])

                # diag weight matrices for the two x-taps, per chunk
                diags = []
                for c in range(2):
                    d0 = sbs.tile([128, 128], bf16, tag=f"d0_{c}")
                    d1 = sbs.tile([128, 128], bf16, tag=f"d1_{c}")
                    nc.vector.tensor_scalar(out=d0[:, :], in0=eye, scalar1=dxw[:, c * 2 : c * 2 + 1], scalar2=None, op0=Alu.mult)
                    nc.vector.tensor_scalar(out=d1[:, :], in0=eye, scalar1=dxw[:, c * 2 + 1 : c * 2 + 2], scalar2=None, op0=Alu.mult)
                    diags.append((d0, d1))

                # ---------- kv gather + k/v projections ----------
                G = sbs.tile([128, 8, 64], f32, tag="G")
                for j in range(8):
                    nc.gpsimd.indirect_dma_start(
                        out=G[:, j, :], out_offset=None, in_=kvT_ap,
                        in_offset=IndirectOffsetOnAxis(ap=idxkv_i[:, j : j + 1], axis=0),
                    )
                xs_b = sbs.tile([65, NS], bf16, tag="xsb")
                nc.vector.memset(xs_b[64:65, :], 1.0)
                for c in range(2):
                    xsT = sbs.tile([128, 64], f32, tag="xsT")
                    nc.vector.tensor_scalar(
                        out=xsT[:, :], in0=G[:, c * 4 + 0, :],
                        scalar1=wkv[:, c * 4 : c * 4 + 1], scalar2=None, op0=Alu.mult,
                    )
                    for t in range(1, 4):
                        nc.vector.scalar_tensor_tensor(
                            out=xsT[:, :], in0=G[:, c * 4 + t, :],
                            scalar=wkv[:, c * 4 + t : c * 4 + t + 1], in1=xsT[:, :],
                            op0=Alu.mult, op1=Alu.add,
                        )
                    xs_p = mps.tile([64, 128], f32, tag="m")
                    nc.tensor.transpose(out=xs_p[:, :], in_=xsT[:, :], identity=eye)
                    nc.vector.tensor_copy(out=xs_b[0:64, c * 128 : (c + 1) * 128], in_=xs_p[:, :])

                if dbg and blk == 0:
                    nc.vector.tensor_copy(out=dbgt[0:1, 2816:3072], in_=xs_b[0:1, :])
                    nc.vector.tensor_copy(out=dbgt[0:1, 3072:3080], in_=idxw[0:1, :])
                    nc.vector.tensor_copy(out=dbgt[0:1, 3080:3082], in_=fyb[0:1, :])
                    nc.vector.tensor_copy(out=dbgt[0:1, 3082:3090], in_=idxkv[0:1, :])
                    nc.vector.tensor_copy(out=dbgt[0:1, 3090:3098], in_=wkv[0:1, :])
                k_p = mps.tile([128, NS], f32, tag="m")
                nc.tensor.matmul(out=k_p[:, :], lhsT=pk_wTs1, rhs=xs_b[:, :], start=True, stop=True)
                k_b = sbs.tile([128, NS], bf16, tag="kb")
                nc.vector.tensor_copy(out=k_b[:, :], in_=k_p[:, :])

                vT1 = sbs.tile([128, 2, 128], bf16, tag="vT1")
                nc.vector.memset(vT1[:, :, :], 0.0)
                nc.vector.memset(vT1[:, :, :].rearrange("p c (h q) -> p c h q", q=32)[:, :, :, 16:17], 1.0)
                for c in range(2):
                    v_p = mps.tile([128, 64], f32, tag="m")
                    nc.tensor.matmul(
                        out=v_p[:, :], lhsT=xs_b[:, c * 128 : (c + 1) * 128], rhs=pv_wT1,
                        start=True, stop=True,
                    )
                    vv = vT1[:, c, :].rearrange("p (h q) -> p h q", q=32)
                    nc.vector.tensor_copy(
                        out=vv[:, :, 0:16],
                        in_=v_p[:, :].rearrange("p (h q) -> p h q", q=16),
                    )

                # ---------- attention per head ----------
                avs = xpool.tile([128, HWS], bf16, tag="avs")
                for h in range(4):
                    P = ppool.tile([128, 2, HWS], bf16, tag="P")
                    for c in range(2):
                        Wt = wpool.tile([128, 4160], bf16, tag="W")
                        nc.gpsimd.indirect_dma_start(
                            out=Wt[:, :], out_offset=None, in_=tab_d,
                            in_offset=IndirectOffsetOnAxis(ap=idxw_i[:, c * 4 + h : c * 4 + h + 1], axis=0),
                        )
                        Dw = wpool.tile([128, 4160], bf16, tag="Dw")
                        nc.gpsimd.indirect_dma_start(
                            out=Dw[:, :], out_offset=None, in_=tab_d,
                            in_offset=IndirectOffsetOnAxis(ap=idxw_i[:, c * 4 + h : c * 4 + h + 1], axis=0),
                            element_offset=NTAB,
                        )
                        Y = wpool.tile([128, 4160], bf16, tag="Y")
                        nc.vector.scalar_tensor_tensor(
                            out=Y[:, :], in0=Dw[:, :], scalar=fyb[:, c : c + 1], in1=Wt[:, :],
                            op0=Alu.mult, op1=Alu.add,
                        )
                        Y3 = Y[:, :].rearrange("p (r q) -> p r q", q=65)
                        d0, d1 = diags[c]
                        kh = k_b[h * 32 : h * 32 + 16, c * 128 : (c + 1) * 128]
                        for wv in range(2):
                            pts = []
                            for i in range(4):
                                mc = wv * 4 + i
                                pt = qkps.tile([128, 512], f32, tag="qkp")
                                nc.tensor.matmul(
                                    out=pt[:, :], lhsT=kh,
                                    rhs=q_b[h * 32 : h * 32 + 16, mc * 512 : (mc + 1) * 512],
                                    start=True, stop=False, tile_position=(h * 32, 0),
                                )
                                pts.append(pt)
                            for i in range(4):
                                mc = wv * 4 + i
                                nc.tensor.matmul(
                                    out=pts[i][:, :], lhsT=d0,
                                    rhs=Y3[:, mc * 8 : (mc + 1) * 8, 0:64],
                                    start=False, stop=False,
                                )
                            for i in range(4):
                                mc = wv * 4 + i
                                nc.tensor.matmul(
                                    out=pts[i][:, :], lhsT=d1,
                                    rhs=Y3[:, mc * 8 : (mc + 1) * 8, 1:65],
                                    start=False, stop=True,
                                )
                            for i in range(4):
                                mc = wv * 4 + i
                                nc.scalar.activation(
                                    out=P[:, c, mc * 512 : (mc + 1) * 512], in_=pts[i][:, :],
                                    func=Act.Exp, bias=zb[:, :],
                                )
                    # AV for this head: [32,512] psum (row 16 = sums), then spread copy
                    for pr in range(4):
                        mca, mcb = pr * 2, pr * 2 + 1
                        a0 = avps.tile([32, 512], f32, tag="avp")
                        a1 = avps.tile([32, 512], f32, tag="avp")
                        for c in range(2):
                            lw = vT1[:, c, h * 32 : (h + 1) * 32]
                            nc.tensor.matmul(out=a0[:, :], lhsT=lw, rhs=P[:, c, mca * 512 : (mca + 1) * 512], start=(c == 0), stop=(c == 1))
                            nc.tensor.matmul(out=a1[:, :], lhsT=lw, rhs=P[:, c, mcb * 512 : (mcb + 1) * 512], start=(c == 0), stop=(c == 1))
                        act_raw(avs[h * 32 : (h + 1) * 32, mca * 512 : (mca + 1) * 512], a0[:, :], Act.Copy)
                        act_raw(avs[h * 32 : (h + 1) * 32, mcb * 512 : (mcb + 1) * 512], a1[:, :], Act.Copy)

                # ---------- normalize + out projection + residual ----------
                for mc in range(8):
                    sb_p = tps.tile([128, 512], f32, tag="tl")
                    nc.tensor.matmul(out=sb_p[:, :], lhsT=b4, rhs=avs[:, mc * 512 : (mc + 1) * 512], start=True, stop=True)
                    rcp = sbs.tile([128, 512], f32, tag="rcp")
                    act_raw(rcp[:, :], sb_p[:, :], Act.Reciprocal)
                    on = sbs.tile([128, 512], bf16, tag="on")
                    nc.vector.tensor_tensor(out=on[:, :], in0=avs[:, mc * 512 : (mc + 1) * 512], in1=rcp[:, :], op=Alu.mult)
                    op = tps.tile([64, 512], f32, tag="tl")
                    nc.tensor.matmul(out=op[:, :], lhsT=po_wT_sp, rhs=on[:, :], start=True, stop=True)
                    nc.vector.scalar_tensor_tensor(
                        out=R[64:128, mc * 512 : (mc + 1) * 512], in0=op[:, :], scalar=po_b_hi,
                        in1=R[64:128, mc * 512 : (mc + 1) * 512], op0=Alu.add, op1=Alu.add,
                    )

            emit_block(0, xi1, kvT0_d, xi1)
            emit_block(1, xi2, kvT0_d, xi2)
            emit_block(2, xi2, kvT1_d, xi2)

            nc.sync.dma_start(out=o1_d, in_=xi1[:, :])
            nc.sync.dma_start(out=o2_d, in_=xi2[:, :])
            if dbg:
                nc.sync.dma_start(out=dbg_d, in_=dbgt[:, :])

    nc.compile()
    return nc


def _host_prep(inputs):
    """Build per-core in_maps. inputs: dict of full numpy arrays."""
    import ml_dtypes

    x0, x1, x2 = inputs["x0"], inputs["x1"], inputs["x2"]

    def spread_cols(m):
        # m: [64(in), 64(out)] -> [64(in), 128] with out col h*16+j at h*32+j
        out = np.zeros((m.shape[0], 128), m.dtype)
        for h in range(4):
            out[:, h * 32 : h * 32 + 16] = m[:, h * 16 : (h + 1) * 16]
        return out

    def spread_rows(v):
        # v: [64, k] -> [128, k] with row h*16+j at h*32+j
        out = np.zeros((128,) + v.shape[1:], v.dtype)
        for h in range(4):
            out[h * 32 : h * 32 + 16] = v[h * 16 : (h + 1) * 16]
        return out

    # weight pack f32: [64, 3*128]  (spread pq_wT)
    wpf = np.zeros((64, 3 * 128), np.float32)
    for b in range(3):
        wpf[:, b * 128 : (b + 1) * 128] = spread_cols(inputs["pq_w"][b].T)
    wpb = np.zeros((65, 3 * 192), ml_dtypes.bfloat16)
    for b in range(3):
        o = b * 192
        pk = np.zeros((65, 128), np.float32)
        pk[0:64] = spread_cols(inputs["pk_w"][b].T * 0.25)
        for h in range(4):
            pk[64, h * 32 : h * 32 + 16] = inputs["pk_b"][b][h * 16 : (h + 1) * 16] * 0.25
        wpb[:, o : o + 128] = pk.astype(ml_dtypes.bfloat16)
        wpb[:64, o + 128 : o + 192] = inputs["pv_w"][b].T.astype(ml_dtypes.bfloat16)
        wpb[64, o + 128 : o + 192] = inputs["pv_b"][b].astype(ml_dtypes.bfloat16)
    # const pack [128, 590]
    cp = np.zeros((128, 590), np.float32)
    cp[:, 0:128] = np.eye(128, dtype=np.float32)
    ys = (np.linspace(0.5, HK - 0.5, HK) / (HK - 1.0)) * 2.0 - 1.0
    cp[0, 128:384] = np.repeat(ys, WK)         # y per n (i-major)
    cp[1, 128:384] = np.tile(ys, HK)           # x per n
    cp[0, 384:512] = 1.0                       # ones1_128
    for h in range(4):
        cp[h * 32 : h * 32 + 16, 520] = 1.0 / 64.0
    for b in range(3):
        cp[:, 521 + b] = spread_rows(inputs["pq_b"][b][:, None])[:, 0]
        cp[64:128, 524 + b] = inputs["po_b"][b]
        bc0 = 527 + b * 21
        cp[:, bc0 : bc0 + 16] = spread_rows(inputs["dw_w"][b].reshape(64, 16))
        cp[:, bc0 + 16] = spread_rows(inputs["dw_b"][b][:, None])[:, 0]
        cp[:, bc0 + 17] = spread_rows(inputs["ln_g"][b][:, None])[:, 0]
        cp[:, bc0 + 18] = spread_rows(inputs["ln_b"][b][:, None])[:, 0]
        cp[:, bc0 + 19 : bc0 + 21] = spread_rows(inputs["pw_w"][b].T)
    cpb = np.zeros((128, 320), ml_dtypes.bfloat16)
    b4 = np.zeros((128, 128), np.float32)
    for h in range(4):
        b4[h * 32 + 16, h * 32 : (h + 1) * 32] = 1.0
    cpb[:, 0:128] = b4.astype(ml_dtypes.bfloat16)
    for b in range(3):
        poT = inputs["po_w"][b].T  # [c, o]
        for h in range(4):
            cpb[h * 32 : h * 32 + 16, 128 + b * 64 : 128 + (b + 1) * 64] = poT[
                h * 16 : (h + 1) * 16
            ].astype(ml_dtypes.bfloat16)
    # rpe slice tables bf16: T windows then D (row-diff) windows
    tab = np.zeros((2, NBLK, NH, 64, TROW, TCOL), ml_dtypes.bfloat16)
    rpe = inputs["rpe"]
    for b in range(3):
        for h in range(4):
            pad = np.zeros((129, 128), np.float32)
            pad[0:127, 0:127] = rpe[b, h]
            dif = pad[1:129] - pad[0:128]
            for x0s in range(64):
                tab[0, b, h, x0s] = pad[0:128, x0s : x0s + 65].astype(ml_dtypes.bfloat16)
                tab[1, b, h, x0s] = dif[:, x0s : x0s + 65].astype(ml_dtypes.bfloat16)
    tab = tab.reshape(-1, 1)

    in_maps = []
    for bb in range(B):
        m = {
            "xi1": np.ascontiguousarray(x1[bb].reshape(C, HWS)),
            "xi2": np.ascontiguousarray(x2[bb].reshape(C, HWS)),
            "kvT0": np.ascontiguousarray(x0[bb, :64].reshape(64, HWS).T),
            "kvT1": np.ascontiguousarray(x1[bb, :64].reshape(64, HWS).T),
            "wpf": wpf,
            "wpb": wpb,
            "cp": cp,
            "cpb": cpb,
            "rpetab": tab,
        }
        in_maps.append(m)
    return in_maps


def kernel(**inputs):
    from concourse.bass_utils import run_bass_kernel_spmd

    if "nc" not in _CACHE:
        _CACHE["nc"] = _build_graph()
    nc = _CACHE["nc"]
    in_maps = _host_prep(inputs)
    res = run_bass_kernel_spmd(nc, in_maps, core_ids=list(range(8)))
    out = np.zeros((NBLK, B, C, H, W), np.float32)
    out[0] = inputs["x0"]
    for bb in range(B):
        out[1, bb] = res.results[bb]["o1"].reshape(C, H, W)
        out[2, bb] = res.results[bb]["o2"].reshape(C, H, W)
    return out



# revision 3
# speedup vs baseline: 1.5647x; 1.5647x over previous
"""Trainium2 Bass kernel for nn_AttentionTD (3-block deformable attention TD).

Self-contained: hardcodes all shapes. Data-parallel over batch B=8 across the
8 NeuronCores; each core runs the full 3-block DAT stack for one batch element.

Pipelined emission: block b+1's front-end (q-proj, offset conv, LN, GELU,
index math, kv gather/proj) is interleaved under block b's attention so the
tensor engine never drains between blocks.
"""

import sys

sys.path.insert(0, "/opt/trn_rl_repo")

import numpy as np

# ---------------- problem constants ----------------
B, C, H, W = 8, 128, 64, 64
NCH = 64          # channels per DAT block
NH, HC = 4, 16    # heads, head channels
KS = 4
HWS = H * W       # 4096
HK = WK = 16
NS = HK * WK      # 256 sample points
EPS = 1e-5
NBLK = 3
# rpe slice table geometry: [blk][h][x0 (64)][row (128)][col (65)]
TROW, TCOL = 128, 65
TSLICE = TROW * TCOL          # 8320
THEAD = 64 * TSLICE           # per (blk,h)
TBLK = NH * THEAD
NTAB = NBLK * TBLK

_CACHE = {}


def _build_graph():
    from concourse import bacc, mybir, tile
    import concourse.bass as bass
    from concourse.bass import IndirectOffsetOnAxis

    f32 = mybir.dt.float32
    bf16 = mybir.dt.bfloat16
    i32 = mybir.dt.int32
    Alu = mybir.AluOpType
    Act = mybir.ActivationFunctionType

    nc = bacc.Bacc("TRN2", target_bir_lowering=False, debug=False, num_devices=8)

    # ---- dram io ----
    xi1_d = nc.dram_tensor("xi1", [C, HWS], f32, kind="ExternalInput").ap()
    xi2_d = nc.dram_tensor("xi2", [C, HWS], f32, kind="ExternalInput").ap()
    kvT0_d = nc.dram_tensor("kvT0", [HWS, NCH], f32, kind="ExternalInput").ap()
    kvT1_d = nc.dram_tensor("kvT1", [HWS, NCH], f32, kind="ExternalInput").ap()
    wpf_d = nc.dram_tensor("wpf", [64, 3 * 128], bf16, kind="ExternalInput").ap()
    wpb_d = nc.dram_tensor("wpb", [65, 3 * 192], bf16, kind="ExternalInput").ap()
    cp_d = nc.dram_tensor("cp", [128, 598], f32, kind="ExternalInput").ap()
    cpb_d = nc.dram_tensor("cpb", [128, 320], bf16, kind="ExternalInput").ap()
    dwd_d = nc.dram_tensor("dwd", [128, 48 * 128], bf16, kind="ExternalInput").ap()
    tab_d = nc.dram_tensor("rpetab", [2 * NTAB, 1], bf16, kind="ExternalInput").ap()
    o1_d = nc.dram_tensor("o1", [C, HWS], f32, kind="ExternalOutput").ap()
    o2_d = nc.dram_tensor("o2", [C, HWS], f32, kind="ExternalOutput").ap()

    with tile.TileContext(nc) as tc:
        import contextlib

        ctx = contextlib.ExitStack()
        with ctx:
            cpool = ctx.enter_context(tc.tile_pool(name="const", bufs=1))
            xpool = ctx.enter_context(tc.tile_pool(name="xdata", bufs=1))
            qpool = ctx.enter_context(tc.tile_pool(name="qtiles", bufs=2))
            wpool = ctx.enter_context(tc.tile_pool(name="wins", bufs=2))
            ppool = ctx.enter_context(tc.tile_pool(name="probs", bufs=1))
            apool = ctx.enter_context(tc.tile_pool(name="avsp", bufs=1))
            spool = ctx.enter_context(tc.tile_pool(name="small", bufs=2))
            qkps = ctx.enter_context(tc.tile_pool(name="qk", bufs=2, space="PSUM"))
            tlps = ctx.enter_context(tc.tile_pool(name="tl", bufs=3, space="PSUM"))
            mps = ctx.enter_context(tc.tile_pool(name="misc", bufs=1, space="PSUM"))

            # ---- persistent loads ----
            cp = cpool.tile([128, 598], f32, tag="cp")
            nc.sync.dma_start(out=cp[:, :], in_=cp_d)
            wpf = cpool.tile([64, 3 * 128], bf16, tag="wpf")
            nc.sync.dma_start(out=wpf[:, :], in_=wpf_d)
            wpb = cpool.tile([65, 3 * 192], bf16, tag="wpb")
            nc.sync.dma_start(out=wpb[:, :], in_=wpb_d)
            cpb = cpool.tile([128, 320], bf16, tag="cpb")
            nc.sync.dma_start(out=cpb[:, :], in_=cpb_d)
            dwd = cpool.tile([128, 48 * 128], bf16, tag="dwd")
            nc.sync.dma_start(out=dwd[:, :], in_=dwd_d)
            xi1 = xpool.tile([C, HWS], f32, tag="xi1")
            nc.sync.dma_start(out=xi1[0:64, :], in_=xi1_d[0:64, :])
            xi2 = xpool.tile([C, HWS], f32, tag="xi2")
            nc.sync.dma_start(out=xi2[0:64, :], in_=xi2_d[0:64, :])
            nc.sync.dma_start(out=xi1[64:128, :], in_=xi1_d[64:128, :])
            nc.sync.dma_start(out=xi2[64:128, :], in_=xi2_d[64:128, :])

            def act_raw(out, in_, func):
                eng = nc.scalar
                ins = [eng.lower_ap(in_)]
                for v in (0.0, 1.0, 0.0):
                    ins.append(mybir.ImmediateValue(dtype=mybir.dt.float32, value=v))
                return eng.add_instruction(
                    mybir.InstActivation(
                        name=nc.get_next_instruction_name(), func=func,
                        ins=ins, outs=[eng.lower_ap(out)],
                    )
                )

            zb = cpool.tile([128, 1], f32, tag="zb")
            nc.vector.memset(zb[:, :], 0.0)
            epst = cpool.tile([1, 1], f32, tag="epst")
            nc.vector.memset(epst[:, :], EPS)
            # vT1 template: zeros with 1.0 at (c*128 + h*32 + 16)
            vtm = cpool.tile([128, 256], bf16, tag="vtm")
            nc.vector.memset(vtm[:, :], 0.0)
            nc.vector.memset(
                vtm[:, :].rearrange("p (c h q) -> p c h q", c=2, q=32)[:, :, :, 16:17],
                1.0,
            )

            eye = cp[:, 0:128]
            ref_yx = cp[0:2, 128:384]          # row0 = y, row1 = x
            ones1_128 = cp[0:1, 384:512]       # [1,128] ones (bcast lhsT)
            ones128_div = cp[0:128, 520:521]   # 1/64 on data rows, 0 on gaps
            kvoff4 = cp[:, 590:594]            # (0,1,64,65) rows
            headoff4 = cp[:, 594:598]          # (0,T,2T,3T) rows

            # bf16 copies of the q sources
            xq1b = xpool.tile([64, HWS], bf16, tag="xq1b")
            nc.vector.tensor_copy(out=xq1b[:, :], in_=xi1[0:64, :])
            xq2b = xpool.tile([64, HWS], bf16, tag="xq2b")
            nc.vector.tensor_copy(out=xq2b[:, :], in_=xi2[0:64, :])

            # ======================= front-end =======================
            def front(blk, XQb, kvT_ap, fc):
                bc0 = 527 + blk * 21
                dw_b = cp[:, bc0 + 16 : bc0 + 17]
                ln_g = cp[:, bc0 + 17 : bc0 + 18]
                ln_b = cp[:, bc0 + 18 : bc0 + 19]
                pw_wT = cp[:, bc0 + 19 : bc0 + 21]
                pq_b_sp = cp[:, 521 + blk : 522 + blk]

                # ---- q projection ----
                q_b = qpool.tile([128, HWS], bf16, tag="qb")
                for mc in range(8):
                    qp = mps.tile([128, 512], f32, tag="m")
                    nc.tensor.matmul(
                        out=qp[:, :], lhsT=wpf[:, blk * 128 : (blk + 1) * 128],
                        rhs=XQb[:, mc * 512 : (mc + 1) * 512], start=True, stop=True,
                    )
                    nc.vector.tensor_scalar(
                        out=q_b[:, mc * 512 : (mc + 1) * 512], in0=qp[:, :],
                        scalar1=pq_b_sp, scalar2=None, op0=Alu.add,
                    )
                    if mc == 3:
                        yield
                fc["q_b"] = q_b
                yield

                # ---- depthwise 4x4 stride-4 conv on tensor engine ----
                q5 = q_b[:, :].rearrange("p (hh a ww b) -> p hh a ww b", hh=16, a=4, ww=16, b=4)
                dwp = mps.tile([128, 256], f32, tag="m")
                for t in range(16):
                    dy, dx = t // 4, t % 4
                    ti = blk * 16 + t
                    nc.tensor.matmul(
                        out=dwp[:, :], lhsT=dwd[:, ti * 128 : (ti + 1) * 128],
                        rhs=q5[:, :, dy, :, dx], start=(t == 0), stop=(t == 15),
                    )
                accp = spool.tile([128, 512], f32, tag="accp")
                nc.scalar.activation(out=accp[:, 0:256], in_=dwp[:, :],
                                     func=Act.Identity, bias=dw_b, scale=1.0)
                nc.vector.tensor_tensor(out=accp[:, 256:512], in0=accp[:, 0:256],
                                        in1=accp[:, 0:256], op=Alu.mult)
                yield

                # ---- layernorm stats ----
                stp = mps.tile([1, 512], f32, tag="m")
                nc.tensor.matmul(out=stp[:, :], lhsT=ones128_div, rhs=accp[:, :],
                                 start=True, stop=True)
                stats = spool.tile([1, 512], f32, tag="stats")
                nc.vector.tensor_copy(out=stats[:, 0:256], in_=stp[:, 0:256])
                var = spool.tile([1, 256], f32, tag="var")
                nc.vector.tensor_tensor(out=var[:, :], in0=stats[:, 0:256],
                                        in1=stats[:, 0:256], op=Alu.mult)
                nc.vector.tensor_tensor(out=var[:, :], in0=stp[:, 256:512],
                                        in1=var[:, :], op=Alu.subtract)
                nc.scalar.activation(out=stats[:, 256:512], in_=var[:, :],
                                     func=Act.Abs_reciprocal_sqrt, bias=epst[:, :])
                yield

                # ---- normalize + gelu ----
                bcp = mps.tile([128, 512], f32, tag="m")
                nc.tensor.matmul(out=bcp[:, :], lhsT=ones1_128, rhs=stats[:, :],
                                 start=True, stop=True)
                t1 = spool.tile([128, 256], f32, tag="t1")
                nc.vector.tensor_tensor(out=t1[:, :], in0=accp[:, 0:256],
                                        in1=bcp[:, 0:256], op=Alu.subtract)
                nc.vector.tensor_tensor(out=t1[:, :], in0=t1[:, :],
                                        in1=bcp[:, 256:512], op=Alu.mult)
                nc.vector.tensor_scalar(out=t1[:, :], in0=t1[:, :], scalar1=ln_g,
                                        scalar2=ln_b, op0=Alu.mult, op1=Alu.add)
                gl = spool.tile([128, 256], f32, tag="gl")
                nc.scalar.activation(out=gl[:, :], in_=t1[:, :], func=Act.Gelu)
                yield

                # ---- offsets -> positions -> posT ----
                offp = mps.tile([2, 256], f32, tag="m")
                nc.tensor.matmul(out=offp[:, :], lhsT=pw_wT, rhs=gl[:, :],
                                 start=True, stop=True)
                pos = spool.tile([2, 256], f32, tag="pos")
                nc.vector.tensor_tensor(out=pos[:, :], in0=offp[:, :], in1=ref_yx, op=Alu.add)
                nc.vector.tensor_scalar(out=pos[:, :], in0=pos[:, :], scalar1=1.0,
                                        scalar2=-1.0, op0=Alu.min, op1=Alu.max)
                posT = spool.tile([128, 4], f32, tag="posT")  # (c0y c0x c1y c1x)
                for c in range(2):
                    tp = mps.tile([128, 2], f32, tag="m")
                    nc.tensor.transpose(out=tp[:, :], in_=pos[:, c * 128 : (c + 1) * 128],
                                        identity=eye[0:2, 0:2])
                    nc.vector.tensor_copy(out=posT[:, c * 2 : c * 2 + 2], in_=tp[:, :])
                yield

                # ---- batched index math ----
                pix = spool.tile([128, 4], f32, tag="pix")
                nc.vector.tensor_scalar(out=pix[:, :], in0=posT[:, :], scalar1=1.0,
                                        scalar2=31.5, op0=Alu.add, op1=Alu.mult)
                rnd = spool.tile([128, 4], f32, tag="rnd")
                nc.vector.tensor_scalar(out=rnd[:, :], in0=pix[:, :], scalar1=8388608.0,
                                        scalar2=-8388608.0, op0=Alu.add, op1=Alu.add)
                gt = spool.tile([128, 4], f32, tag="gt")
                nc.vector.tensor_tensor(out=gt[:, :], in0=rnd[:, :], in1=pix[:, :], op=Alu.is_gt)
                p0 = spool.tile([128, 4], f32, tag="p0")
                nc.vector.tensor_tensor(out=p0[:, :], in0=rnd[:, :], in1=gt[:, :], op=Alu.subtract)
                nc.vector.tensor_scalar(out=p0[:, :], in0=p0[:, :], scalar1=62.0,
                                        scalar2=None, op0=Alu.min)
                fr = spool.tile([128, 4], f32, tag="fr")
                nc.vector.tensor_tensor(out=fr[:, :], in0=pix[:, :], in1=p0[:, :], op=Alu.subtract)
                fr1 = spool.tile([128, 4], f32, tag="fr1")
                nc.vector.tensor_scalar(out=fr1[:, :], in0=fr[:, :], scalar1=-1.0,
                                        scalar2=1.0, op0=Alu.mult, op1=Alu.add)
                fc["fr"] = fr
                fc["fr1"] = fr1
                yield

                # chunk views: v=0 -> y, v=1 -> x
                p0v = p0[:, :].rearrange("p (c v) -> p c v", v=2)
                frv = fr[:, :].rearrange("p (c v) -> p c v", v=2)
                fr1v = fr1[:, :].rearrange("p (c v) -> p c v", v=2)

                # ---- kv bilinear weights + gather indices ----
                wkv = spool.tile([128, 8], f32, tag="wkv")
                wkv4 = wkv[:, :].rearrange("p (c t) -> p c t", t=4)
                nc.vector.tensor_tensor(out=wkv4[:, :, 0], in0=fr1v[:, :, 0], in1=fr1v[:, :, 1], op=Alu.mult)
                nc.vector.tensor_tensor(out=wkv4[:, :, 1], in0=fr1v[:, :, 0], in1=frv[:, :, 1], op=Alu.mult)
                nc.vector.tensor_tensor(out=wkv4[:, :, 2], in0=frv[:, :, 0], in1=fr1v[:, :, 1], op=Alu.mult)
                nc.vector.tensor_tensor(out=wkv4[:, :, 3], in0=frv[:, :, 0], in1=frv[:, :, 1], op=Alu.mult)
                ib = spool.tile([128, 2], f32, tag="ib")
                nc.vector.scalar_tensor_tensor(out=ib[:, :], in0=p0v[:, :, 0], scalar=64.0,
                                               in1=p0v[:, :, 1], op0=Alu.mult, op1=Alu.add)
                idxkv = spool.tile([128, 8], f32, tag="idxkv")
                for c in range(2):
                    nc.vector.tensor_tensor(
                        out=idxkv[:, c * 4 : (c + 1) * 4],
                        in0=ib[:, c : c + 1].to_broadcast([128, 4]),
                        in1=kvoff4, op=Alu.add,
                    )
                idxkv_i = spool.tile([128, 8], i32, tag="idxkvi")
                nc.vector.tensor_copy(out=idxkv_i[:, :], in_=idxkv[:, :])
                G = spool.tile([128, 8, 64], f32, tag="G")
                for j in range(8):
                    nc.gpsimd.indirect_dma_start(
                        out=G[:, j, :], out_offset=None, in_=kvT_ap,
                        in_offset=IndirectOffsetOnAxis(ap=idxkv_i[:, j : j + 1], axis=0),
                    )
                yield

                # ---- bias-window indices + diag weights ----
                q0b = spool.tile([128, 4], f32, tag="q0b")
                nc.vector.tensor_scalar(out=q0b[:, :], in0=p0[:, :], scalar1=-1.0,
                                        scalar2=62.0, op0=Alu.mult, op1=Alu.add)
                q0bv = q0b[:, :].rearrange("p (c v) -> p c v", v=2)
                iw = spool.tile([128, 2], f32, tag="iw")
                nc.vector.scalar_tensor_tensor(out=iw[:, :], in0=q0bv[:, :, 1], scalar=128.0,
                                               in1=q0bv[:, :, 0], op0=Alu.mult, op1=Alu.add)
                nc.vector.tensor_scalar(out=iw[:, :], in0=iw[:, :], scalar1=65.0,
                                        scalar2=float(blk * TBLK), op0=Alu.mult, op1=Alu.add)
                idxw = spool.tile([128, 8], f32, tag="idxw")
                for c in range(2):
                    nc.vector.tensor_tensor(
                        out=idxw[:, c * 4 : (c + 1) * 4],
                        in0=iw[:, c : c + 1].to_broadcast([128, 4]),
                        in1=headoff4, op=Alu.add,
                    )
                idxw_i = spool.tile([128, 8], i32, tag="idxwi")
                nc.vector.tensor_copy(out=idxw_i[:, :], in_=idxw[:, :])
                fc["idxw_i"] = idxw_i
                # diag weight matrices: d0 <- fx, d1 <- 1-fx (per chunk)
                diags = []
                for c in range(2):
                    d0 = spool.tile([128, 128], bf16, tag=f"d0_{c}")
                    d1 = spool.tile([128, 128], bf16, tag=f"d1_{c}")
                    nc.vector.tensor_scalar(out=d0[:, :], in0=eye,
                                            scalar1=fr[:, c * 2 + 1 : c * 2 + 2],
                                            scalar2=None, op0=Alu.mult)
                    nc.vector.tensor_scalar(out=d1[:, :], in0=eye,
                                            scalar1=fr1[:, c * 2 + 1 : c * 2 + 2],
                                            scalar2=None, op0=Alu.mult)
                    diags.append((d0, d1))
                fc["diags"] = diags
                yield

                # ---- gathered kv -> xs ----
                xs_b = spool.tile([65, 256], bf16, tag="xsb")
                nc.vector.memset(xs_b[64:65, :], 1.0)
                for c in range(2):
                    xsT = spool.tile([128, 64], f32, tag="xsT")
                    nc.vector.tensor_scalar(
                        out=xsT[:, :], in0=G[:, c * 4 + 0, :],
                        scalar1=wkv[:, c * 4 : c * 4 + 1], scalar2=None, op0=Alu.mult,
                    )
                    for t in range(1, 4):
                        nc.vector.scalar_tensor_tensor(
                            out=xsT[:, :], in0=G[:, c * 4 + t, :],
                            scalar=wkv[:, c * 4 + t : c * 4 + t + 1], in1=xsT[:, :],
                            op0=Alu.mult, op1=Alu.add,
                        )
                    xsp = mps.tile([64, 128], f32, tag="m")
                    nc.tensor.transpose(out=xsp[:, :], in_=xsT[:, :], identity=eye)
                    nc.scalar.activation(out=xs_b[0:64, c * 128 : (c + 1) * 128],
                                         in_=xsp[:, :], func=Act.Copy)
                    if c == 0:
                        yield
                yield

                # ---- k / v projections ----
                kp = mps.tile([128, 256], f32, tag="m")
                nc.tensor.matmul(out=kp[:, :], lhsT=wpb[0:65, blk * 192 : blk * 192 + 128],
                                 rhs=xs_b[:, :], start=True, stop=True)
                k_b = spool.tile([128, 256], bf16, tag="kb")
                nc.scalar.activation(out=k_b[:, :], in_=kp[:, :], func=Act.Copy)
                fc["k_b"] = k_b
                vT1 = spool.tile([128, 256], bf16, tag="vT1")
                nc.vector.tensor_copy(out=vT1[:, :], in_=vtm[:, :])
                for c in range(2):
                    vp = mps.tile([128, 64], f32, tag="m")
                    nc.tensor.matmul(
                        out=vp[:, :], lhsT=xs_b[:, c * 128 : (c + 1) * 128],
                        rhs=wpb[0:65, blk * 192 + 128 : blk * 192 + 192],
                        start=True, stop=True,
                    )
                    vv = vT1[:, c * 128 : (c + 1) * 128].rearrange("p (h q) -> p h q", q=32)
                    nc.scalar.activation(
                        out=vv[:, :, 0:16],
                        in_=vp[:, :].rearrange("p (h q) -> p h q", q=16),
                        func=Act.Copy,
                    )
                fc["vT1"] = vT1
                yield

            # ======================= attention =======================
            def attn(blk, fc, R, feeder):
                po_wT_sp = cpb[:, 128 + blk * 64 : 128 + (blk + 1) * 64]
                b4 = cpb[:, 0:128]
                po_b_hi = cp[64:128, 524 + blk : 525 + blk]
                q_b = fc["q_b"]
                k_b = fc["k_b"]
                vT1 = fc["vT1"]
                idxw_i = fc["idxw_i"]
                fr1 = fc["fr1"]
                diags = fc["diags"]

                steps = [(h, c) for h in range(4) for c in range(2)]

                def issue_gather(i):
                    h, c = steps[i]
                    Wt = wpool.tile([128, 4160], bf16, tag="Wt", bufs=3)
                    nc.gpsimd.indirect_dma_start(
                        out=Wt[:, :], out_offset=None, in_=tab_d,
                        in_offset=IndirectOffsetOnAxis(
                            ap=idxw_i[:, c * 4 + h : c * 4 + h + 1], axis=0),
                    )
                    Dw = wpool.tile([128, 4160], bf16, tag="Dw", bufs=3)
                    nc.gpsimd.indirect_dma_start(
                        out=Dw[:, :], out_offset=None, in_=tab_d,
                        in_offset=IndirectOffsetOnAxis(
                            ap=idxw_i[:, c * 4 + h : c * 4 + h + 1], axis=0),
                        element_offset=NTAB,
                    )
                    return Wt, Dw

                avs = apool.tile([128, HWS], bf16, tag="avs")
                pend = {i: issue_gather(i) for i in range(3)}
                P = None
                for i, (h, c) in enumerate(steps):
                    if c == 0:
                        P = ppool.tile([128, 2, HWS], bf16, tag="P")
                    Wt, Dw = pend.pop(i)
                    if i + 3 < 8:
                        pend[i + 3] = issue_gather(i + 3)
                    # y-interp: Y = Wt + (1-fy) * Dw
                    Y = wpool.tile([128, 4160], bf16, tag="Y")
                    nc.vector.tensor_scalar(out=Y[:, :], in0=Dw[:, :],
                                            scalar1=fr1[:, c * 2 : c * 2 + 1],
                                            scalar2=None, op0=Alu.mult)
                    nc.vector.tensor_tensor(out=Y[:, :], in0=Y[:, :], in1=Wt[:, :], op=Alu.add)
                    Y3 = Y[:, :].rearrange("p (r q) -> p r q", q=65)
                    kh = k_b[h * 32 : h * 32 + 16, c * 128 : (c + 1) * 128]
                    d0, d1 = diags[c]
                    for k in range(4):
                        qk = qkps.tile([128, 1024], f32, tag="qk")
                        for hf in range(2):
                            mc = k * 2 + hf
                            nc.tensor.matmul(
                                out=qk[:, hf * 512 : (hf + 1) * 512], lhsT=kh,
                                rhs=q_b[h * 32 : h * 32 + 16, mc * 512 : (mc + 1) * 512],
                                start=True, stop=False, tile_position=(h * 32, 0),
                            )
                        for hf in range(2):
                            mc = k * 2 + hf
                            nc.tensor.matmul(
                                out=qk[:, hf * 512 : (hf + 1) * 512], lhsT=d0,
                                rhs=Y3[:, mc * 8 : (mc + 1) * 8, 0:64],
                                start=False, stop=False,
                            )
                        for hf in range(2):
                            mc = k * 2 + hf
                            nc.tensor.matmul(
                                out=qk[:, hf * 512 : (hf + 1) * 512], lhsT=d1,
                                rhs=Y3[:, mc * 8 : (mc + 1) * 8, 1:65],
                                start=False, stop=True,
                            )
                        nc.scalar.activation(
                            out=P[:, c, k * 1024 : (k + 1) * 1024], in_=qk[:, :],
                            func=Act.Exp, bias=zb[:, :],
                        )
                    feeder()
                    if c == 1:
                        # AV for this head
                        for pr in range(4):
                            a0 = tlps.tile([128, 512], f32, tag="tl")
                            a1 = tlps.tile([128, 512], f32, tag="tl")
                            mca, mcb = pr * 2, pr * 2 + 1
                            for cc in range(2):
                                lw = vT1[:, cc * 128 + h * 32 : cc * 128 + (h + 1) * 32]
                                nc.tensor.matmul(
                                    out=a0[0:32, :], lhsT=lw,
                                    rhs=P[:, cc, mca * 512 : (mca + 1) * 512],
                                    start=(cc == 0), stop=(cc == 1),
                                )
                                nc.tensor.matmul(
                                    out=a1[0:32, :], lhsT=lw,
                                    rhs=P[:, cc, mcb * 512 : (mcb + 1) * 512],
                                    start=(cc == 0), stop=(cc == 1),
                                )
                            nc.vector.tensor_copy(
                                out=avs[h * 32 : (h + 1) * 32, mca * 512 : (mca + 1) * 512],
                                in_=a0[0:32, :])
                            nc.vector.tensor_copy(
                                out=avs[h * 32 : (h + 1) * 32, mcb * 512 : (mcb + 1) * 512],
                                in_=a1[0:32, :])
                        feeder()

                # ---- normalize + out projection + residual ----
                for mc in range(8):
                    sbp = tlps.tile([128, 512], f32, tag="tl")
                    nc.tensor.matmul(out=sbp[:, :], lhsT=b4,
                                     rhs=avs[:, mc * 512 : (mc + 1) * 512],
                                     start=True, stop=True)
                    rcp = spool.tile([128, 512], f32, tag="rcp")
                    act_raw(rcp[:, :], sbp[:, :], Act.Reciprocal)
                    on = spool.tile([128, 512], bf16, tag="on")
                    nc.vector.tensor_tensor(out=on[:, :],
                                            in0=avs[:, mc * 512 : (mc + 1) * 512],
                                            in1=rcp[:, :], op=Alu.mult)
                    op = tlps.tile([128, 512], f32, tag="tl")
                    nc.tensor.matmul(out=op[0:64, :], lhsT=po_wT_sp, rhs=on[:, :],
                                     start=True, stop=True)
                    nc.vector.scalar_tensor_tensor(
                        out=R[64:128, mc * 512 : (mc + 1) * 512], in0=op[0:64, :],
                        scalar=po_b_hi, in1=R[64:128, mc * 512 : (mc + 1) * 512],
                        op0=Alu.add, op1=Alu.add,
                    )
                    if mc % 2 == 1:
                        feeder()

            def make_feeder(gen):
                def feeder():
                    if gen is None:
                        return
                    try:
                        next(gen)
                    except StopIteration:
                        pass
                return feeder

            def drain(gen):
                for _ in gen:
                    pass

            # ======================= schedule =======================
            fc0 = {}
            drain(front(0, xq1b, kvT0_d, fc0))
            fc1 = {}
            g1 = front(1, xq2b, kvT0_d, fc1)
            attn(0, fc0, xi1, make_feeder(g1))
            nc.sync.dma_start(out=o1_d, in_=xi1[:, :])
            drain(g1)
            fc2 = {}
            g2 = front(2, xq2b, kvT1_d, fc2)
            attn(1, fc1, xi2, make_feeder(g2))
            drain(g2)
            attn(2, fc2, xi2, make_feeder(None))
            nc.sync.dma_start(out=o2_d, in_=xi2[:, :])

    nc.compile()
    return nc


def _host_prep(inputs):
    """Build per-core in_maps. inputs: dict of full numpy arrays."""
    import ml_dtypes

    x0, x1, x2 = inputs["x0"], inputs["x1"], inputs["x2"]

    def spread_cols(m):
        # m: [64(in), 64(out)] -> [64(in), 128] with out col h*16+j at h*32+j
        out = np.zeros((m.shape[0], 128), m.dtype)
        for h in range(4):
            out[:, h * 32 : h * 32 + 16] = m[:, h * 16 : (h + 1) * 16]
        return out

    def spread_rows(v):
        # v: [64, k] -> [128, k] with row h*16+j at h*32+j
        out = np.zeros((128,) + v.shape[1:], v.dtype)
        for h in range(4):
            out[h * 32 : h * 32 + 16] = v[h * 16 : (h + 1) * 16]
        return out

    # weight pack bf16: [64, 3*128]  (spread pq_wT)
    wpf = np.zeros((64, 3 * 128), np.float32)
    for b in range(3):
        wpf[:, b * 128 : (b + 1) * 128] = spread_cols(inputs["pq_w"][b].T)
    wpf = wpf.astype(ml_dtypes.bfloat16)
    wpb = np.zeros((65, 3 * 192), ml_dtypes.bfloat16)
    for b in range(3):
        o = b * 192
        pk = np.zeros((65, 128), np.float32)
        pk[0:64] = spread_cols(inputs["pk_w"][b].T * 0.25)
        for h in range(4):
            pk[64, h * 32 : h * 32 + 16] = inputs["pk_b"][b][h * 16 : (h + 1) * 16] * 0.25
        wpb[:, o : o + 128] = pk.astype(ml_dtypes.bfloat16)
        wpb[:64, o + 128 : o + 192] = inputs["pv_w"][b].T.astype(ml_dtypes.bfloat16)
        wpb[64, o + 128 : o + 192] = inputs["pv_b"][b].astype(ml_dtypes.bfloat16)
    # const pack [128, 598]
    cp = np.zeros((128, 598), np.float32)
    cp[:, 0:128] = np.eye(128, dtype=np.float32)
    ys = (np.linspace(0.5, HK - 0.5, HK) / (HK - 1.0)) * 2.0 - 1.0
    cp[0, 128:384] = np.repeat(ys, WK)         # y per n (i-major)
    cp[1, 128:384] = np.tile(ys, HK)           # x per n
    cp[0, 384:512] = 1.0                       # ones1_128
    for h in range(4):
        cp[h * 32 : h * 32 + 16, 520] = 1.0 / 64.0
    for b in range(3):
        cp[:, 521 + b] = spread_rows(inputs["pq_b"][b][:, None])[:, 0]
        cp[64:128, 524 + b] = inputs["po_b"][b]
        bc0 = 527 + b * 21
        cp[:, bc0 + 16] = spread_rows(inputs["dw_b"][b][:, None])[:, 0]
        cp[:, bc0 + 17] = spread_rows(inputs["ln_g"][b][:, None])[:, 0]
        cp[:, bc0 + 18] = spread_rows(inputs["ln_b"][b][:, None])[:, 0]
        cp[:, bc0 + 19 : bc0 + 21] = spread_rows(inputs["pw_w"][b].T)
    cp[:, 590] = 0.0
    cp[:, 591] = 1.0
    cp[:, 592] = 64.0
    cp[:, 593] = 65.0
    for h in range(4):
        cp[:, 594 + h] = float(h * THEAD)
    cpb = np.zeros((128, 320), ml_dtypes.bfloat16)
    b4 = np.zeros((128, 128), np.float32)
    for h in range(4):
        b4[h * 32 + 16, h * 32 : (h + 1) * 32] = 1.0
    cpb[:, 0:128] = b4.astype(ml_dtypes.bfloat16)
    for b in range(3):
        poT = inputs["po_w"][b].T  # [c, o]
        for h in range(4):
            cpb[h * 32 : h * 32 + 16, 128 + b * 64 : 128 + (b + 1) * 64] = poT[
                h * 16 : (h + 1) * 16
            ].astype(ml_dtypes.bfloat16)
    # depthwise conv taps as spread diagonal matrices [128,128] bf16
    dwd = np.zeros((128, 48 * 128), np.float32)
    for b in range(3):
        dwsp = spread_rows(inputs["dw_w"][b].reshape(64, 16))  # [128, 16]
        for t in range(16):
            ti = b * 16 + t
            np.fill_diagonal(dwd[:, ti * 128 : (ti + 1) * 128], dwsp[:, t])
    dwd = dwd.astype(ml_dtypes.bfloat16)
    # rpe slice tables bf16: T windows then D (row-diff) windows
    tab = np.zeros((2, NBLK, NH, 64, TROW, TCOL), ml_dtypes.bfloat16)
    rpe = inputs["rpe"]
    for b in range(3):
        for h in range(4):
            pad = np.zeros((129, 128), np.float32)
            pad[0:127, 0:127] = rpe[b, h]
            dif = pad[1:129] - pad[0:128]
            for x0s in range(64):
                tab[0, b, h, x0s] = pad[0:128, x0s : x0s + 65].astype(ml_dtypes.bfloat16)
                tab[1, b, h, x0s] = dif[:, x0s : x0s + 65].astype(ml_dtypes.bfloat16)
    tab = tab.reshape(-1, 1)

    in_maps = []
    for bb in range(B):
        m = {
            "xi1": np.ascontiguousarray(x1[bb].reshape(C, HWS)),
            "xi2": np.ascontiguousarray(x2[bb].reshape(C, HWS)),
            "kvT0": np.ascontiguousarray(x0[bb, :64].reshape(64, HWS).T),
            "kvT1": np.ascontiguousarray(x1[bb, :64].reshape(64, HWS).T),
            "wpf": wpf,
            "wpb": wpb,
            "cp": cp,
            "cpb": cpb,
            "dwd": dwd,
            "rpetab": tab,
        }
        in_maps.append(m)
    return in_maps


def kernel(**inputs):
    from concourse.bass_utils import run_bass_kernel_spmd

    if "nc" not in _CACHE:
        _CACHE["nc"] = _build_graph()
    nc = _CACHE["nc"]
    in_maps = _host_prep(inputs)
    res = run_bass_kernel_spmd(nc, in_maps, core_ids=list(range(8)))
    out = np.zeros((NBLK, B, C, H, W), np.float32)
    out[0] = inputs["x0"]
    for bb in range(B):
        out[1, bb] = res.results[bb]["o1"].reshape(C, H, W)
        out[2, bb] = res.results[bb]["o2"].reshape(C, H, W)
    return out


# revision 4
# speedup vs baseline: 1.5731x; 1.0054x over previous
"""Trainium2 Bass kernel for nn_AttentionTD (3-block deformable attention TD).

Self-contained: hardcodes all shapes. Data-parallel over batch B=8 across the
8 NeuronCores; each core runs the full 3-block DAT stack for one batch element.

Pipelined emission: block b+1's front-end (q-proj, offset conv, LN, GELU,
index math, kv gather/proj) is interleaved under block b's attention so the
tensor engine never drains between blocks.
"""

import sys

sys.path.insert(0, "/opt/trn_rl_repo")

import numpy as np

# ---------------- problem constants ----------------
B, C, H, W = 8, 128, 64, 64
NCH = 64          # channels per DAT block
NH, HC = 4, 16    # heads, head channels
KS = 4
HWS = H * W       # 4096
HK = WK = 16
NS = HK * WK      # 256 sample points
EPS = 1e-5
NBLK = 3
# rpe slice table geometry: [blk][h][x0 (64)][row (128)][col (65)]
TROW, TCOL = 128, 65
TSLICE = TROW * TCOL          # 8320
THEAD = 64 * TSLICE           # per (blk,h)
TBLK = NH * THEAD
NTAB = NBLK * TBLK

_CACHE = {}


def _build_graph():
    from concourse import bacc, mybir, tile
    import concourse.bass as bass
    from concourse.bass import IndirectOffsetOnAxis

    f32 = mybir.dt.float32
    bf16 = mybir.dt.bfloat16
    i32 = mybir.dt.int32
    Alu = mybir.AluOpType
    Act = mybir.ActivationFunctionType

    nc = bacc.Bacc("TRN2", target_bir_lowering=False, debug=False, num_devices=8)

    # ---- dram io ----
    xi1_d = nc.dram_tensor("xi1", [C, HWS], f32, kind="ExternalInput").ap()
    xi2_d = nc.dram_tensor("xi2", [C, HWS], f32, kind="ExternalInput").ap()
    kvT0_d = nc.dram_tensor("kvT0", [HWS, NCH], f32, kind="ExternalInput").ap()
    kvT1_d = nc.dram_tensor("kvT1", [HWS, NCH], f32, kind="ExternalInput").ap()
    wpf_d = nc.dram_tensor("wpf", [64, 3 * 128], f32, kind="ExternalInput").ap()
    wpb_d = nc.dram_tensor("wpb", [65, 3 * 192], bf16, kind="ExternalInput").ap()
    cp_d = nc.dram_tensor("cp", [128, 598], f32, kind="ExternalInput").ap()
    cpb_d = nc.dram_tensor("cpb", [128, 320], bf16, kind="ExternalInput").ap()
    dwd_d = nc.dram_tensor("dwd", [128, 48 * 128], bf16, kind="ExternalInput").ap()
    tab_d = nc.dram_tensor("rpetab", [2 * NTAB, 1], bf16, kind="ExternalInput").ap()
    o1_d = nc.dram_tensor("o1", [C, HWS], f32, kind="ExternalOutput").ap()
    o2_d = nc.dram_tensor("o2", [C, HWS], f32, kind="ExternalOutput").ap()

    with tile.TileContext(nc) as tc:
        import contextlib

        ctx = contextlib.ExitStack()
        with ctx:
            cpool = ctx.enter_context(tc.tile_pool(name="const", bufs=1))
            xpool = ctx.enter_context(tc.tile_pool(name="xdata", bufs=1))
            qpool = ctx.enter_context(tc.tile_pool(name="qtiles", bufs=2))
            wpool = ctx.enter_context(tc.tile_pool(name="wins", bufs=2))
            ppool = ctx.enter_context(tc.tile_pool(name="probs", bufs=1))
            apool = ctx.enter_context(tc.tile_pool(name="avsp", bufs=2))
            spool = ctx.enter_context(tc.tile_pool(name="small", bufs=2))
            qkps = ctx.enter_context(tc.tile_pool(name="qk", bufs=2, space="PSUM"))
            tlps = ctx.enter_context(tc.tile_pool(name="tl", bufs=3, space="PSUM"))
            mps = ctx.enter_context(tc.tile_pool(name="misc", bufs=1, space="PSUM"))

            # ---- persistent loads ----
            cp = cpool.tile([128, 598], f32, tag="cp")
            nc.sync.dma_start(out=cp[:, :], in_=cp_d)
            wpf = cpool.tile([64, 3 * 128], f32, tag="wpf")
            nc.sync.dma_start(out=wpf[:, :], in_=wpf_d)
            wpb = cpool.tile([65, 3 * 192], bf16, tag="wpb")
            nc.sync.dma_start(out=wpb[:, :], in_=wpb_d)
            cpb = cpool.tile([128, 320], bf16, tag="cpb")
            nc.sync.dma_start(out=cpb[:, :], in_=cpb_d)
            dwd = cpool.tile([128, 48 * 128], bf16, tag="dwd")
            nc.sync.dma_start(out=dwd[:, :], in_=dwd_d)
            xi1 = xpool.tile([C, HWS], f32, tag="xi1")
            nc.sync.dma_start(out=xi1[0:64, :], in_=xi1_d[0:64, :])
            xi2 = xpool.tile([C, HWS], f32, tag="xi2")
            nc.sync.dma_start(out=xi2[0:64, :], in_=xi2_d[0:64, :])
            nc.sync.dma_start(out=xi1[64:128, :], in_=xi1_d[64:128, :])
            nc.sync.dma_start(out=xi2[64:128, :], in_=xi2_d[64:128, :])

            def act_raw(out, in_, func):
                eng = nc.scalar
                ins = [eng.lower_ap(in_)]
                for v in (0.0, 1.0, 0.0):
                    ins.append(mybir.ImmediateValue(dtype=mybir.dt.float32, value=v))
                return eng.add_instruction(
                    mybir.InstActivation(
                        name=nc.get_next_instruction_name(), func=func,
                        ins=ins, outs=[eng.lower_ap(out)],
                    )
                )

            zb = cpool.tile([128, 1], f32, tag="zb")
            nc.vector.memset(zb[:, :], 0.0)
            epst = cpool.tile([1, 1], f32, tag="epst")
            nc.vector.memset(epst[:, :], EPS)
            # vT1 template: zeros with 1.0 at (c*128 + h*32 + 16)
            vtm = cpool.tile([128, 256], bf16, tag="vtm")
            nc.vector.memset(vtm[:, :], 0.0)
            nc.vector.memset(
                vtm[:, :].rearrange("p (c h q) -> p c h q", c=2, q=32)[:, :, :, 16:17],
                1.0,
            )

            eye = cp[:, 0:128]
            ref_yx = cp[0:2, 128:384]          # row0 = y, row1 = x
            ones1_128 = cp[0:1, 384:512]       # [1,128] ones (bcast lhsT)
            ones128_div = cp[0:128, 520:521]   # 1/64 on data rows, 0 on gaps
            kvoff4 = cp[:, 590:594]            # (0,1,64,65) rows
            headoff4 = cp[:, 594:598]          # (0,T,2T,3T) rows


            # ======================= front-end =======================
            def front(blk, XQb, kvT_ap, fc):
                bc0 = 527 + blk * 21
                dw_b = cp[:, bc0 + 16 : bc0 + 17]
                ln_g = cp[:, bc0 + 17 : bc0 + 18]
                ln_b = cp[:, bc0 + 18 : bc0 + 19]
                pw_wT = cp[:, bc0 + 19 : bc0 + 21]
                pq_b_sp = cp[:, 521 + blk : 522 + blk]

                # ---- q projection ----
                q_b = qpool.tile([128, HWS], bf16, tag="qb")
                for mc in range(8):
                    qp = mps.tile([128, 512], f32, tag="m")
                    nc.tensor.matmul(
                        out=qp[:, :], lhsT=wpf[:, blk * 128 : (blk + 1) * 128],
                        rhs=XQb[:, mc * 512 : (mc + 1) * 512], start=True, stop=True,
                    )
                    nc.vector.tensor_scalar(
                        out=q_b[:, mc * 512 : (mc + 1) * 512], in0=qp[:, :],
                        scalar1=pq_b_sp, scalar2=None, op0=Alu.add,
                    )
                    if mc == 3:
                        yield
                fc["q_b"] = q_b
                yield

                # ---- depthwise 4x4 stride-4 conv on tensor engine ----
                q5 = q_b[:, :].rearrange("p (hh a ww b) -> p hh a ww b", hh=16, a=4, ww=16, b=4)
                dwp = mps.tile([128, 256], f32, tag="m")
                for t in range(16):
                    dy, dx = t // 4, t % 4
                    ti = blk * 16 + t
                    nc.tensor.matmul(
                        out=dwp[:, :], lhsT=dwd[:, ti * 128 : (ti + 1) * 128],
                        rhs=q5[:, :, dy, :, dx], start=(t == 0), stop=(t == 15),
                    )
                accp = spool.tile([128, 512], f32, tag="accp")
                nc.scalar.activation(out=accp[:, 0:256], in_=dwp[:, :],
                                     func=Act.Identity, bias=dw_b, scale=1.0)
                nc.vector.tensor_tensor(out=accp[:, 256:512], in0=accp[:, 0:256],
                                        in1=accp[:, 0:256], op=Alu.mult)
                yield

                # ---- layernorm stats ----
                stp = mps.tile([1, 512], f32, tag="m")
                nc.tensor.matmul(out=stp[:, :], lhsT=ones128_div, rhs=accp[:, :],
                                 start=True, stop=True)
                stats = spool.tile([1, 512], f32, tag="stats")
                nc.vector.tensor_copy(out=stats[:, 0:256], in_=stp[:, 0:256])
                var = spool.tile([1, 256], f32, tag="var")
                nc.vector.tensor_tensor(out=var[:, :], in0=stats[:, 0:256],
                                        in1=stats[:, 0:256], op=Alu.mult)
                nc.vector.tensor_tensor(out=var[:, :], in0=stp[:, 256:512],
                                        in1=var[:, :], op=Alu.subtract)
                nc.scalar.activation(out=stats[:, 256:512], in_=var[:, :],
                                     func=Act.Abs_reciprocal_sqrt, bias=epst[:, :])
                yield

                # ---- normalize + gelu ----
                bcp = mps.tile([128, 512], f32, tag="m")
                nc.tensor.matmul(out=bcp[:, :], lhsT=ones1_128, rhs=stats[:, :],
                                 start=True, stop=True)
                t1 = spool.tile([128, 256], f32, tag="t1")
                nc.vector.tensor_tensor(out=t1[:, :], in0=accp[:, 0:256],
                                        in1=bcp[:, 0:256], op=Alu.subtract)
                nc.vector.tensor_tensor(out=t1[:, :], in0=t1[:, :],
                                        in1=bcp[:, 256:512], op=Alu.mult)
                nc.vector.tensor_scalar(out=t1[:, :], in0=t1[:, :], scalar1=ln_g,
                                        scalar2=ln_b, op0=Alu.mult, op1=Alu.add)
                gl = spool.tile([128, 256], f32, tag="gl")
                nc.scalar.activation(out=gl[:, :], in_=t1[:, :], func=Act.Gelu)
                yield

                # ---- offsets -> positions -> posT ----
                offp = mps.tile([2, 256], f32, tag="m")
                nc.tensor.matmul(out=offp[:, :], lhsT=pw_wT, rhs=gl[:, :],
                                 start=True, stop=True)
                pos = spool.tile([2, 256], f32, tag="pos")
                nc.vector.tensor_tensor(out=pos[:, :], in0=offp[:, :], in1=ref_yx, op=Alu.add)
                nc.vector.tensor_scalar(out=pos[:, :], in0=pos[:, :], scalar1=1.0,
                                        scalar2=-1.0, op0=Alu.min, op1=Alu.max)
                posT = spool.tile([128, 4], f32, tag="posT")  # (c0y c0x c1y c1x)
                for c in range(2):
                    tp = mps.tile([128, 2], f32, tag="m")
                    nc.tensor.transpose(out=tp[:, :], in_=pos[:, c * 128 : (c + 1) * 128],
                                        identity=eye[0:2, 0:2])
                    nc.vector.tensor_copy(out=posT[:, c * 2 : c * 2 + 2], in_=tp[:, :])
                yield

                # ---- batched index math ----
                pix = spool.tile([128, 4], f32, tag="pix")
                nc.vector.tensor_scalar(out=pix[:, :], in0=posT[:, :], scalar1=1.0,
                                        scalar2=31.5, op0=Alu.add, op1=Alu.mult)
                rnd = spool.tile([128, 4], f32, tag="rnd")
                nc.vector.tensor_scalar(out=rnd[:, :], in0=pix[:, :], scalar1=8388608.0,
                                        scalar2=-8388608.0, op0=Alu.add, op1=Alu.add)
                gt = spool.tile([128, 4], f32, tag="gt")
                nc.vector.tensor_tensor(out=gt[:, :], in0=rnd[:, :], in1=pix[:, :], op=Alu.is_gt)
                p0 = spool.tile([128, 4], f32, tag="p0")
                nc.vector.tensor_tensor(out=p0[:, :], in0=rnd[:, :], in1=gt[:, :], op=Alu.subtract)
                nc.vector.tensor_scalar(out=p0[:, :], in0=p0[:, :], scalar1=62.0,
                                        scalar2=None, op0=Alu.min)
                fr = spool.tile([128, 4], f32, tag="fr")
                nc.vector.tensor_tensor(out=fr[:, :], in0=pix[:, :], in1=p0[:, :], op=Alu.subtract)
                fr1 = spool.tile([128, 4], f32, tag="fr1")
                nc.vector.tensor_scalar(out=fr1[:, :], in0=fr[:, :], scalar1=-1.0,
                                        scalar2=1.0, op0=Alu.mult, op1=Alu.add)
                fc["fr"] = fr
                fc["fr1"] = fr1
                yield

                # chunk views: v=0 -> y, v=1 -> x
                p0v = p0[:, :].rearrange("p (c v) -> p c v", v=2)
                frv = fr[:, :].rearrange("p (c v) -> p c v", v=2)
                fr1v = fr1[:, :].rearrange("p (c v) -> p c v", v=2)

                # ---- kv bilinear weights + gather indices ----
                wkv = spool.tile([128, 8], f32, tag="wkv")
                wkv4 = wkv[:, :].rearrange("p (c t) -> p c t", t=4)
                nc.vector.tensor_tensor(out=wkv4[:, :, 0], in0=fr1v[:, :, 0], in1=fr1v[:, :, 1], op=Alu.mult)
                nc.vector.tensor_tensor(out=wkv4[:, :, 1], in0=fr1v[:, :, 0], in1=frv[:, :, 1], op=Alu.mult)
                nc.vector.tensor_tensor(out=wkv4[:, :, 2], in0=frv[:, :, 0], in1=fr1v[:, :, 1], op=Alu.mult)
                nc.vector.tensor_tensor(out=wkv4[:, :, 3], in0=frv[:, :, 0], in1=frv[:, :, 1], op=Alu.mult)
                ib = spool.tile([128, 2], f32, tag="ib")
                nc.vector.scalar_tensor_tensor(out=ib[:, :], in0=p0v[:, :, 0], scalar=64.0,
                                               in1=p0v[:, :, 1], op0=Alu.mult, op1=Alu.add)
                idxkv = spool.tile([128, 8], f32, tag="idxkv")
                for c in range(2):
                    nc.vector.tensor_tensor(
                        out=idxkv[:, c * 4 : (c + 1) * 4],
                        in0=ib[:, c : c + 1].to_broadcast([128, 4]),
                        in1=kvoff4, op=Alu.add,
                    )
                idxkv_i = spool.tile([128, 8], i32, tag="idxkvi")
                nc.vector.tensor_copy(out=idxkv_i[:, :], in_=idxkv[:, :])
                G = spool.tile([128, 8, 64], f32, tag="G")
                for j in range(8):
                    nc.gpsimd.indirect_dma_start(
                        out=G[:, j, :], out_offset=None, in_=kvT_ap,
                        in_offset=IndirectOffsetOnAxis(ap=idxkv_i[:, j : j + 1], axis=0),
                    )
                yield

                # ---- bias-window indices + diag weights ----
                q0b = spool.tile([128, 4], f32, tag="q0b")
                nc.vector.tensor_scalar(out=q0b[:, :], in0=p0[:, :], scalar1=-1.0,
                                        scalar2=62.0, op0=Alu.mult, op1=Alu.add)
                q0bv = q0b[:, :].rearrange("p (c v) -> p c v", v=2)
                iw = spool.tile([128, 2], f32, tag="iw")
                nc.vector.scalar_tensor_tensor(out=iw[:, :], in0=q0bv[:, :, 1], scalar=128.0,
                                               in1=q0bv[:, :, 0], op0=Alu.mult, op1=Alu.add)
                nc.vector.tensor_scalar(out=iw[:, :], in0=iw[:, :], scalar1=65.0,
                                        scalar2=float(blk * TBLK), op0=Alu.mult, op1=Alu.add)
                idxw = spool.tile([128, 8], f32, tag="idxw")
                for c in range(2):
                    nc.vector.tensor_tensor(
                        out=idxw[:, c * 4 : (c + 1) * 4],
                        in0=iw[:, c : c + 1].to_broadcast([128, 4]),
                        in1=headoff4, op=Alu.add,
                    )
                idxw_i = spool.tile([128, 8], i32, tag="idxwi")
                nc.vector.tensor_copy(out=idxw_i[:, :], in_=idxw[:, :])
                fc["idxw_i"] = idxw_i
                # diag weight matrices: d0 <- fx, d1 <- 1-fx (per chunk)
                diags = []
                for c in range(2):
                    d0 = spool.tile([128, 128], bf16, tag=f"d0_{c}")
                    d1 = spool.tile([128, 128], bf16, tag=f"d1_{c}")
                    nc.vector.tensor_scalar(out=d0[:, :], in0=eye,
                                            scalar1=fr[:, c * 2 + 1 : c * 2 + 2],
                                            scalar2=None, op0=Alu.mult)
                    nc.vector.tensor_scalar(out=d1[:, :], in0=eye,
                                            scalar1=fr1[:, c * 2 + 1 : c * 2 + 2],
                                            scalar2=None, op0=Alu.mult)
                    diags.append((d0, d1))
                fc["diags"] = diags
                yield

                # ---- gathered kv -> xs ----
                xs_b = spool.tile([65, 256], bf16, tag="xsb")
                nc.vector.memset(xs_b[64:65, :], 1.0)
                for c in range(2):
                    xsT = spool.tile([128, 64], f32, tag="xsT")
                    nc.vector.tensor_scalar(
                        out=xsT[:, :], in0=G[:, c * 4 + 0, :],
                        scalar1=wkv[:, c * 4 : c * 4 + 1], scalar2=None, op0=Alu.mult,
                    )
                    for t in range(1, 4):
                        nc.vector.scalar_tensor_tensor(
                            out=xsT[:, :], in0=G[:, c * 4 + t, :],
                            scalar=wkv[:, c * 4 + t : c * 4 + t + 1], in1=xsT[:, :],
                            op0=Alu.mult, op1=Alu.add,
                        )
                    xsp = mps.tile([64, 128], f32, tag="m")
                    nc.tensor.transpose(out=xsp[:, :], in_=xsT[:, :], identity=eye)
                    nc.scalar.activation(out=xs_b[0:64, c * 128 : (c + 1) * 128],
                                         in_=xsp[:, :], func=Act.Copy)
                    if c == 0:
                        yield
                yield

                # ---- k / v projections ----
                kp = mps.tile([128, 256], f32, tag="m")
                nc.tensor.matmul(out=kp[:, :], lhsT=wpb[0:65, blk * 192 : blk * 192 + 128],
                                 rhs=xs_b[:, :], start=True, stop=True)
                k_b = spool.tile([128, 256], bf16, tag="kb")
                nc.scalar.activation(out=k_b[:, :], in_=kp[:, :], func=Act.Copy)
                fc["k_b"] = k_b
                vT1 = spool.tile([128, 256], bf16, tag="vT1")
                nc.vector.tensor_copy(out=vT1[:, :], in_=vtm[:, :])
                for c in range(2):
                    vp = mps.tile([128, 64], f32, tag="m")
                    nc.tensor.matmul(
                        out=vp[:, :], lhsT=xs_b[:, c * 128 : (c + 1) * 128],
                        rhs=wpb[0:65, blk * 192 + 128 : blk * 192 + 192],
                        start=True, stop=True,
                    )
                    vv = vT1[:, c * 128 : (c + 1) * 128].rearrange("p (h q) -> p h q", q=32)
                    nc.scalar.activation(
                        out=vv[:, :, 0:16],
                        in_=vp[:, :].rearrange("p (h q) -> p h q", q=16),
                        func=Act.Copy,
                    )
                fc["vT1"] = vT1
                yield

            # ======================= attention =======================
            def attn(blk, fc, R, feeder):
                po_wT_sp = cpb[:, 128 + blk * 64 : 128 + (blk + 1) * 64]
                b4 = cpb[:, 0:128]
                po_b_hi = cp[64:128, 524 + blk : 525 + blk]
                q_b = fc["q_b"]
                k_b = fc["k_b"]
                vT1 = fc["vT1"]
                idxw_i = fc["idxw_i"]
                fr1 = fc["fr1"]
                diags = fc["diags"]

                steps = [(h, c) for h in range(4) for c in range(2)]

                def issue_gather(i):
                    h, c = steps[i]
                    Wt = wpool.tile([128, 4160], bf16, tag="Wt", bufs=3)
                    nc.gpsimd.indirect_dma_start(
                        out=Wt[:, :], out_offset=None, in_=tab_d,
                        in_offset=IndirectOffsetOnAxis(
                            ap=idxw_i[:, c * 4 + h : c * 4 + h + 1], axis=0),
                    )
                    Dw = wpool.tile([128, 4160], bf16, tag="Dw", bufs=3)
                    nc.gpsimd.indirect_dma_start(
                        out=Dw[:, :], out_offset=None, in_=tab_d,
                        in_offset=IndirectOffsetOnAxis(
                            ap=idxw_i[:, c * 4 + h : c * 4 + h + 1], axis=0),
                        element_offset=NTAB,
                    )
                    return Wt, Dw

                avs = apool.tile([128, HWS], bf16, tag="avs")
                pend = {i: issue_gather(i) for i in range(3)}
                P = None
                for i, (h, c) in enumerate(steps):
                    if c == 0:
                        P = ppool.tile([128, 2, HWS], bf16, tag="P")
                    Wt, Dw = pend.pop(i)
                    if i + 3 < 8:
                        pend[i + 3] = issue_gather(i + 3)
                    # y-interp: Y = Wt + (1-fy) * Dw
                    Y = wpool.tile([128, 4160], bf16, tag="Y")
                    nc.vector.tensor_scalar(out=Y[:, :], in0=Dw[:, :],
                                            scalar1=fr1[:, c * 2 : c * 2 + 1],
                                            scalar2=None, op0=Alu.mult)
                    nc.vector.tensor_tensor(out=Y[:, :], in0=Y[:, :], in1=Wt[:, :], op=Alu.add)
                    Y3 = Y[:, :].rearrange("p (r q) -> p r q", q=65)
                    kh = k_b[h * 32 : h * 32 + 16, c * 128 : (c + 1) * 128]
                    d0, d1 = diags[c]
                    for k in range(4):
                        qk = qkps.tile([128, 1024], f32, tag="qk")
                        for hf in range(2):
                            mc = k * 2 + hf
                            nc.tensor.matmul(
                                out=qk[:, hf * 512 : (hf + 1) * 512], lhsT=kh,
                                rhs=q_b[h * 32 : h * 32 + 16, mc * 512 : (mc + 1) * 512],
                                start=True, stop=False, tile_position=(h * 32, 0),
                            )
                        for hf in range(2):
                            mc = k * 2 + hf
                            nc.tensor.matmul(
                                out=qk[:, hf * 512 : (hf + 1) * 512], lhsT=d0,
                                rhs=Y3[:, mc * 8 : (mc + 1) * 8, 0:64],
                                start=False, stop=False,
                            )
                        for hf in range(2):
                            mc = k * 2 + hf
                            nc.tensor.matmul(
                                out=qk[:, hf * 512 : (hf + 1) * 512], lhsT=d1,
                                rhs=Y3[:, mc * 8 : (mc + 1) * 8, 1:65],
                                start=False, stop=True,
                            )
                        nc.scalar.activation(
                            out=P[:, c, k * 1024 : (k + 1) * 1024], in_=qk[:, :],
                            func=Act.Exp, bias=zb[:, :],
                        )
                    feeder()
                    if c == 1:
                        # AV for this head
                        for pr in range(4):
                            a0 = tlps.tile([128, 512], f32, tag="tl")
                            a1 = tlps.tile([128, 512], f32, tag="tl")
                            mca, mcb = pr * 2, pr * 2 + 1
                            for cc in range(2):
                                lw = vT1[:, cc * 128 + h * 32 : cc * 128 + (h + 1) * 32]
                                nc.tensor.matmul(
                                    out=a0[0:32, :], lhsT=lw,
                                    rhs=P[:, cc, mca * 512 : (mca + 1) * 512],
                                    start=(cc == 0), stop=(cc == 1),
                                )
                                nc.tensor.matmul(
                                    out=a1[0:32, :], lhsT=lw,
                                    rhs=P[:, cc, mcb * 512 : (mcb + 1) * 512],
                                    start=(cc == 0), stop=(cc == 1),
                                )
                            nc.vector.tensor_copy(
                                out=avs[h * 32 : (h + 1) * 32, mca * 512 : (mca + 1) * 512],
                                in_=a0[0:32, :])
                            nc.vector.tensor_copy(
                                out=avs[h * 32 : (h + 1) * 32, mcb * 512 : (mcb + 1) * 512],
                                in_=a1[0:32, :])
                        feeder()

                # ---- normalize + out projection + residual ----
                for mc in range(8):
                    sbp = tlps.tile([128, 512], f32, tag="tl")
                    nc.tensor.matmul(out=sbp[:, :], lhsT=b4,
                                     rhs=avs[:, mc * 512 : (mc + 1) * 512],
                                     start=True, stop=True)
                    rcp = spool.tile([128, 512], f32, tag="rcp")
                    act_raw(rcp[:, :], sbp[:, :], Act.Reciprocal)
                    on = spool.tile([128, 512], bf16, tag="on")
                    nc.vector.tensor_tensor(out=on[:, :],
                                            in0=avs[:, mc * 512 : (mc + 1) * 512],
                                            in1=rcp[:, :], op=Alu.mult)
                    op = tlps.tile([128, 512], f32, tag="tl")
                    nc.tensor.matmul(out=op[0:64, :], lhsT=po_wT_sp, rhs=on[:, :],
                                     start=True, stop=True)
                    nc.vector.scalar_tensor_tensor(
                        out=R[64:128, mc * 512 : (mc + 1) * 512], in0=op[0:64, :],
                        scalar=po_b_hi, in1=R[64:128, mc * 512 : (mc + 1) * 512],
                        op0=Alu.add, op1=Alu.add,
                    )
                    if mc % 2 == 1:
                        feeder()

            def make_feeder(gen):
                def feeder():
                    if gen is None:
                        return
                    try:
                        next(gen)
                    except StopIteration:
                        pass
                return feeder

            def drain(gen):
                for _ in gen:
                    pass

            # ======================= schedule =======================
            fc0 = {}
            drain(front(0, xi1[0:64, :], kvT0_d, fc0))
            fc1 = {}
            g1 = front(1, xi2[0:64, :], kvT0_d, fc1)
            attn(0, fc0, xi1, make_feeder(g1))
            nc.sync.dma_start(out=o1_d, in_=xi1[:, :])
            drain(g1)
            fc2 = {}
            g2 = front(2, xi2[0:64, :], kvT1_d, fc2)
            attn(1, fc1, xi2, make_feeder(g2))
            drain(g2)
            attn(2, fc2, xi2, make_feeder(None))
            nc.sync.dma_start(out=o2_d, in_=xi2[:, :])

    nc.compile()
    return nc


def _host_prep(inputs):
    """Build per-core in_maps. inputs: dict of full numpy arrays."""
    import ml_dtypes

    x0, x1, x2 = inputs["x0"], inputs["x1"], inputs["x2"]

    def spread_cols(m):
        # m: [64(in), 64(out)] -> [64(in), 128] with out col h*16+j at h*32+j
        out = np.zeros((m.shape[0], 128), m.dtype)
        for h in range(4):
            out[:, h * 32 : h * 32 + 16] = m[:, h * 16 : (h + 1) * 16]
        return out

    def spread_rows(v):
        # v: [64, k] -> [128, k] with row h*16+j at h*32+j
        out = np.zeros((128,) + v.shape[1:], v.dtype)
        for h in range(4):
            out[h * 32 : h * 32 + 16] = v[h * 16 : (h + 1) * 16]
        return out

    # weight pack bf16: [64, 3*128]  (spread pq_wT)
    wpf = np.zeros((64, 3 * 128), np.float32)
    for b in range(3):
        wpf[:, b * 128 : (b + 1) * 128] = spread_cols(inputs["pq_w"][b].T)
    wpb = np.zeros((65, 3 * 192), ml_dtypes.bfloat16)
    for b in range(3):
        o = b * 192
        pk = np.zeros((65, 128), np.float32)
        pk[0:64] = spread_cols(inputs["pk_w"][b].T * 0.25)
        for h in range(4):
            pk[64, h * 32 : h * 32 + 16] = inputs["pk_b"][b][h * 16 : (h + 1) * 16] * 0.25
        wpb[:, o : o + 128] = pk.astype(ml_dtypes.bfloat16)
        wpb[:64, o + 128 : o + 192] = inputs["pv_w"][b].T.astype(ml_dtypes.bfloat16)
        wpb[64, o + 128 : o + 192] = inputs["pv_b"][b].astype(ml_dtypes.bfloat16)
    # const pack [128, 598]
    cp = np.zeros((128, 598), np.float32)
    cp[:, 0:128] = np.eye(128, dtype=np.float32)
    ys = (np.linspace(0.5, HK - 0.5, HK) / (HK - 1.0)) * 2.0 - 1.0
    cp[0, 128:384] = np.repeat(ys, WK)         # y per n (i-major)
    cp[1, 128:384] = np.tile(ys, HK)           # x per n
    cp[0, 384:512] = 1.0                       # ones1_128
    for h in range(4):
        cp[h * 32 : h * 32 + 16, 520] = 1.0 / 64.0
    for b in range(3):
        cp[:, 521 + b] = spread_rows(inputs["pq_b"][b][:, None])[:, 0]
        cp[64:128, 524 + b] = inputs["po_b"][b]
        bc0 = 527 + b * 21
        cp[:, bc0 + 16] = spread_rows(inputs["dw_b"][b][:, None])[:, 0]
        cp[:, bc0 + 17] = spread_rows(inputs["ln_g"][b][:, None])[:, 0]
        cp[:, bc0 + 18] = spread_rows(inputs["ln_b"][b][:, None])[:, 0]
        cp[:, bc0 + 19 : bc0 + 21] = spread_rows(inputs["pw_w"][b].T)
    cp[:, 590] = 0.0
    cp[:, 591] = 1.0
    cp[:, 592] = 64.0
    cp[:, 593] = 65.0
    for h in range(4):
        cp[:, 594 + h] = float(h * THEAD)
    cpb = np.zeros((128, 320), ml_dtypes.bfloat16)
    b4 = np.zeros((128, 128), np.float32)
    for h in range(4):
        b4[h * 32 + 16, h * 32 : (h + 1) * 32] = 1.0
    cpb[:, 0:128] = b4.astype(ml_dtypes.bfloat16)
    for b in range(3):
        poT = inputs["po_w"][b].T  # [c, o]
        for h in range(4):
            cpb[h * 32 : h * 32 + 16, 128 + b * 64 : 128 + (b + 1) * 64] = poT[
                h * 16 : (h + 1) * 16
            ].astype(ml_dtypes.bfloat16)
    # depthwise conv taps as spread diagonal matrices [128,128] bf16
    dwd = np.zeros((128, 48 * 128), np.float32)
    for b in range(3):
        dwsp = spread_rows(inputs["dw_w"][b].reshape(64, 16))  # [128, 16]
        for t in range(16):
            ti = b * 16 + t
            np.fill_diagonal(dwd[:, ti * 128 : (ti + 1) * 128], dwsp[:, t])
    dwd = dwd.astype(ml_dtypes.bfloat16)
    # rpe slice tables bf16: T windows then D (row-diff) windows
    tab = np.zeros((2, NBLK, NH, 64, TROW, TCOL), ml_dtypes.bfloat16)
    rpe = inputs["rpe"]
    for b in range(3):
        for h in range(4):
            pad = np.zeros((129, 128), np.float32)
            pad[0:127, 0:127] = rpe[b, h]
            dif = pad[1:129] - pad[0:128]
            for x0s in range(64):
                tab[0, b, h, x0s] = pad[0:128, x0s : x0s + 65].astype(ml_dtypes.bfloat16)
                tab[1, b, h, x0s] = dif[:, x0s : x0s + 65].astype(ml_dtypes.bfloat16)
    tab = tab.reshape(-1, 1)

    in_maps = []
    for bb in range(B):
        m = {
            "xi1": np.ascontiguousarray(x1[bb].reshape(C, HWS)),
            "xi2": np.ascontiguousarray(x2[bb].reshape(C, HWS)),
            "kvT0": np.ascontiguousarray(x0[bb, :64].reshape(64, HWS).T),
            "kvT1": np.ascontiguousarray(x1[bb, :64].reshape(64, HWS).T),
            "wpf": wpf,
            "wpb": wpb,
            "cp": cp,
            "cpb": cpb,
            "dwd": dwd,
            "rpetab": tab,
        }
        in_maps.append(m)
    return in_maps


def kernel(**inputs):
    from concourse.bass_utils import run_bass_kernel_spmd

    if "nc" not in _CACHE:
        _CACHE["nc"] = _build_graph()
    nc = _CACHE["nc"]
    in_maps = _host_prep(inputs)
    res = run_bass_kernel_spmd(nc, in_maps, core_ids=list(range(8)))
    out = np.zeros((NBLK, B, C, H, W), np.float32)
    out[0] = inputs["x0"]
    for bb in range(B):
        out[1, bb] = res.results[bb]["o1"].reshape(C, H, W)
        out[2, bb] = res.results[bb]["o2"].reshape(C, H, W)
    return out
